# revision 46
# baseline (speedup 1.0000x reference)
"""Trainium2 Bass kernel for nn_LSTMEncoder: 5-layer bidirectional LSTM (B=16,L=64,H=400)
+ pairwise quintic-poly MLP head, algebraically collapsed.

Sharding: 8 cores = 2 directions x 4 batch-groups (B=4/core). Direction is encoded in
per-core DATA (weights/masks/index order), program is identical (SPMD).

Per-layer dir-pair exchange: masked-staging pair ReduceScatter delivering only the
PARTNER's hidden states, split in two halves — the first half fires mid-recurrence
(t 0..31 final after step 31) and hides under the remaining steps. The next layer's
input transform is split into an own-dir phase (runs during the exchange, straight
from local xout) and a partner phase (time-chunked in reverse so the rows the next
recurrence reads first are evacuated first). The MLP head computes its own-direction
poly/matmul work under the final exchange; all head matmuls run in fp16.
"""
import numpy as np
from contextlib import ExitStack

import concourse.bass as bass
import concourse.bacc as bacc
import concourse.tile as tile
from concourse import mybir
from concourse.bass_utils import run_bass_kernel_spmd

F32 = mybir.dt.float32
F16 = mybir.dt.float16
F8 = mybir.dt.float8e4
NP_F8 = mybir.dt.np(F8)
AF = mybir.ActivationFunctionType
ALU = mybir.AluOpType

H = 400
L = 64          # seq len / steps
B = 16          # total batch
BC = 4          # batch per core
NL = 5
NCORES = 8
GATE_SRC = [0, 1, 3, 2]   # q order (i,f,o,g) -> original gate block (i,f,g,o)

# ---------------- M-tile geometry ----------------
# 16 M-tiles: m<12 -> (q=m//3, k=m%3), 128 rows; m>=12 -> q=m-12, k=3, 16 rows.
def mtile_info(m):
    if m < 12:
        q, k = divmod(m, 3)
        return q * 16 + k * 4, 128, (q * 3 + k) * 128, q, k
    q = m - 12
    return q * 16 + 12, 16, 1536 + q * 16, q, 3


def _col_order():
    """order[j] = original Whh row index placed at lhsT free-col j."""
    order = []
    for q in range(4):
        for k in range(3):
            for r in range(128):
                order.append(GATE_SRC[q] * 400 + k * 128 + r)
    for q in range(4):
        for r in range(16):
            order.append(GATE_SRC[q] * 400 + 384 + r)
    return np.array(order)

COL_ORDER = _col_order()


def _prep_lhsT(W, nhalves):
    """W: (1600, D) with D = 400*nhalves. Returns (4*nhalves, 128, 1600) fp16 lhsT tiles.
    Rows (contraction) are split into nhalves halves of 400, each zero-padded to 512."""
    Wr = W[COL_ORDER, :]                       # (1600, D) reordered gate rows
    halves = []
    for s in range(nhalves):
        h = Wr[:, s * 400:(s + 1) * 400]       # (1600, 400)
        h = np.concatenate([h, np.zeros((1600, 112), h.dtype)], axis=1)  # pad to 512
        halves.append(h)
    Wp = np.concatenate(halves, axis=1)        # (1600, 512*nh)
    lhsT = Wp.T.reshape(4 * nhalves, 128, 1600)
    return np.ascontiguousarray(lhsT.astype(np.float16))


def _prep_bias(bvec):
    """(1600,) -> (16,128) f32 per-M-tile per-partition bias."""
    b = bvec[COL_ORDER]
    out = np.zeros((16, 128), np.float32)
    for m in range(16):
        pc, rows, wc, q, k = mtile_info(m)
        out[m, :rows] = b[wc:wc + rows]
    return out


def _prep_w1h(W1h):
    """W1h: (100, 400) -> (4,128,100) fp16 lhsT tiles (400 rows padded to 512)."""
    T = W1h.T  # (400, 100)
    Tp = np.concatenate([T, np.zeros((112, 100), T.dtype)], 0)
    return np.ascontiguousarray(Tp.reshape(4, 128, 100).astype(np.float16))


# ---------------- device program ----------------
def build_program():
    nc = bacc.Bacc("TRN2", target_bir_lowering=False, debug=False, num_devices=NCORES)
    dp = nc.declare_dram_parameter
    x0T_d = dp("x0T", [128, 16 * L], F16, isOutput=False)
    whh_d = dp("whhT", [NL, 4, 128, 1600], F16, isOutput=False)
    wih0_d = dp("wih0T", [4, 128, 1600], F16, isOutput=False)
    wihr_d = dp("wihrT", [NL - 1, 8, 128, 1600], F16, isOutput=False)
    bias_d = dp("bias", [NL, 16, 128], F32, isOutput=False)
    masks_d = dp("masks", [4, 128], F32, isOutput=False)
    consts_d = dp("consts", [8, 128], F32, isOutput=False)
    w1ao_d = dp("w1aT_own", [4, 128, 100], F16, isOutput=False)
    w1ap_d = dp("w1aT_part", [4, 128, 100], F16, isOutput=False)
    w1bo_d = dp("w1bT_own", [4, 128, 100], F16, isOutput=False)
    w1bp_d = dp("w1bT_part", [4, 128, 100], F16, isOutput=False)
    out_d = dp("out", [32, 256], F32, isOutput=True)

    groups = [[g, g + 4] for g in range(4)]

    with tile.TileContext(nc) as tc, ExitStack() as ctx:
        pool1 = ctx.enter_context(tc.tile_pool(name="persist", bufs=1))
        whhp = ctx.enter_context(tc.tile_pool(name="whh", bufs=2))
        wihp = ctx.enter_context(tc.tile_pool(name="wih", bufs=2))
        xsp = ctx.enter_context(tc.tile_pool(name="xs", bufs=1))
        xop = ctx.enter_context(tc.tile_pool(name="xout", bufs=2))
        xip = ctx.enter_context(tc.tile_pool(name="xin", bufs=2))
        gp = ctx.enter_context(tc.tile_pool(name="gates", bufs=3))
        sp = ctx.enter_context(tc.tile_pool(name="small", bufs=4))
        php = ctx.enter_context(tc.tile_pool(name="phase", bufs=1))
        gps = ctx.enter_context(tc.tile_pool(name="gpsum", bufs=2, space="PSUM"))
        xps = ctx.enter_context(tc.tile_pool(name="xpsum", bufs=2, space="PSUM"))
        pps = ctx.enter_context(tc.tile_pool(name="ppsum", bufs=2, space="PSUM"))
        drp = ctx.enter_context(tc.tile_pool(name="dram", bufs=2, space="DRAM"))

        # ---- persistent loads (x0T + wih0 first: they gate the first matmuls) ----
        x0T = pool1.tile([128, 16 * L], F16, tag="x0T")
        nc.sync.dma_start(x0T[:], x0T_d[:, :])
        # persistent xs buffer (memset once; garbage cols stay 0)
        xs_own = pool1.tile([128, L * 64], F32, tag="xs_own")
        nc.vector.memset(xs_own[:], 0.0)
        # zero the gate-psum slots once (garbage lanes read later must be finite 0)
        z = gps.tile([128, 32], F32, tag="gps_if", bufs=1)
        nc.vector.memset(z[:], 0.0)
        z2 = gps.tile([128, 32], F32, tag="gps_og", bufs=1)
        nc.vector.memset(z2[:], 0.0)

        def xs_matmul(lhsT_sb, pairs, lay, own, t0=0, t1=L):
            """pairs: (lhsT k-tile idx, rhs [p,t,4] view); contracts rhs[:, t0:t1].
            own=True: xs_own[t0:t1] = bias + contrib (overwrite);
            own=False: xs_own[L-t1:L-t0] += contrib reversed in t."""
            xov = xs_own[:].rearrange("p (t c) -> p t c", c=64)
            nt = t1 - t0
            for m in range(16):
                pc, rows, wc, q, k0 = mtile_info(m)
                ps = xps.tile([128, 256], F32, tag="xps")
                for i, (ki, rhs) in enumerate(pairs):
                    lw = lhsT_sb[:, ki * 1600 + wc: ki * 1600 + wc + rows]
                    nc.tensor.matmul(ps[:rows, :nt * 4], lw[:, :], rhs[:, t0:t1, :],
                                     start=(i == 0), stop=(i == len(pairs) - 1))
                pv = ps[:rows, :nt * 4].rearrange("p (t b) -> p t b", b=4)
                if own:
                    biasap = bias_sb[:rows, lay * 16 + m: lay * 16 + m + 1]
                    nc.vector.tensor_scalar(xov[:rows, t0:t1, pc:pc + 4], pv, biasap, None,
                                            op0=ALU.add)
                else:
                    nc.vector.tensor_add(xov[:rows, L - t1:L - t0, pc:pc + 4],
                                         xov[:rows, L - t1:L - t0, pc:pc + 4],
                                         pv[:, ::-1, :])

        # ---- layer 0 xs ----
        x0v = x0T[:].rearrange("p (t c) -> p t c", c=16)
        wih0_sb = wihp.tile([128, 4 * 1600], F16, tag="wih", bufs=1)
        for k in range(4):
            nc.sync.dma_start(wih0_sb[:, k * 1600:(k + 1) * 1600], wih0_d.ap()[k])
        bias_sb = pool1.tile([128, NL * 16], F32, tag="bias")
        nc.sync.dma_start(bias_sb[:], bias_d.ap().rearrange("l m p -> p (l m)"))
        masks_sb = pool1.tile([128, 4], F32, tag="masks")
        nc.sync.dma_start(masks_sb[:], masks_d.ap().rearrange("m p -> p m"))
        consts_sb = pool1.tile([128, 8], F32, tag="consts")
        nc.sync.dma_start(consts_sb[:], consts_d.ap().rearrange("m p -> p m"))
        for t0 in range(0, L, 16):  # t-chunked so the layer-0 loop starts early
            xs_matmul(wih0_sb, [(k, x0v[:, :, k * 4:(k + 1) * 4]) for k in range(4)],
                      0, True, t0, t0 + 16)

        def load_whh(l):
            t = whhp.tile([128, 4 * 1600], F16, tag="whh", name=f"whh{l}")
            nc.sync.dma_start(t[:].rearrange("p (k n) -> p k n", n=1600),
                              whh_d.ap()[l].rearrange("k p n -> p k n"))
            return t

        def load_wihr(l):
            t = wihp.tile([128, 8 * 1600], F16, tag="wihr", name=f"wihr{l}")
            nc.sync.dma_start(t[:].rearrange("p (k n) -> p k n", n=1600),
                              wihr_d.ap()[l - 1].rearrange("k p n -> p k n"))
            return t

        # m-tile emission order: i,f gate tiles first (unlocks sigmoid(i,f)
        # while g,o tiles still stream), then g, then o.
        M_IF = [0, 1, 2, 3, 4, 5, 12, 13]
        M_G = [9, 10, 11, 15]
        M_O = [6, 7, 8, 14]

        whh_l = load_whh(0)
        wihr_next = load_wihr(1)

        # head weights (per-core own/part halves, fp16); loaded during layer 0
        w1_sb = {}
        for nm, dd in (("ao", w1ao_d), ("ap", w1ap_d), ("bo", w1bo_d), ("bp", w1bp_d)):
            wt = pool1.tile([128, 400], F16, tag="w1" + nm)
            nc.sync.dma_start(wt[:].rearrange("p (k n) -> p k n", n=100),
                              dd.ap().rearrange("k p n -> p k n"))
            w1_sb[nm] = wt
        consts16 = pool1.tile([128, 8], F16, tag="c16")
        nc.vector.tensor_copy(consts16[:], consts_sb[:])

        def poly_eval(dst, x_ap):
            """dst = x*(w1 + x*(w2 + x*(w3 + x*(w4 + x*w5)))) elementwise."""
            nc.vector.tensor_scalar(dst, x_ap, consts_sb[:, 4:5], consts_sb[:, 3:4],
                                    op0=ALU.mult, op1=ALU.add)
            for wi in (2, 1, 0):
                nc.vector.tensor_mul(dst, dst, x_ap)
                nc.vector.tensor_scalar(dst, dst, consts_sb[:, wi:wi + 1], None,
                                        op0=ALU.add)
            nc.vector.tensor_mul(dst, dst, x_ap)

        def halfmat(w_sb, Pv, ps):
            for j in range(4):
                nc.tensor.matmul(ps[:100, :], w_sb[:, j * 100:(j + 1) * 100],
                                 Pv[:, :, j * 4:j * 4 + 4], start=(j == 0), stop=(j == 3))

        A_sb = php.tile([128, 256], F32, tag="A")
        C_sb = php.tile([128, 256], F16, tag="C")

        for l in range(NL):
            if l > 0:
                whh_l = load_whh(l)
            xout = xop.tile([128, 16 * L], F16, tag="xout")
            xov_ = xout[:].rearrange("p (t c) -> p t c", c=16)
            c_t = sp.tile([128, 16], F32, tag="c")
            nc.vector.memset(c_t[:], 0.0)
            # exchange is split in halves: the first fires mid-loop (t 0..31
            # of xout are final after step 31) and hides under steps 32..63
            stg = xip.tile([128, 2048], F16, tag="stg")
            in_b1 = drp.tile([2, 128, 512], F16, tag="arin")
            out_b1 = drp.tile([128, 512], F16, tag="arout")
            in_b2 = drp.tile([2, 128, 512], F16, tag="arin")
            out_b2 = drp.tile([128, 512], F16, tag="arout")
            xpart = xip.tile([128, 1024], F16, tag="xpart")
            for t in range(L):
                if t == 32:
                    nc.vector.tensor_scalar_mul(stg[:, 0:512], xout[:, 0:512],
                                                masks_sb[:, 2:3])
                    nc.gpsimd.tensor_scalar_mul(stg[:, 1024:1536], xout[:, 0:512],
                                                masks_sb[:, 3:4])
                    for j in range(2):
                        nc.sync.dma_start(in_b1[:][j], stg[:, j * 1024:j * 1024 + 512])
                    nc.gpsimd.collective_compute(
                        "ReduceScatter", ALU.add, ins=[in_b1[:].opt()],
                        outs=[out_b1[:].opt()], replica_groups=groups)
                    nc.sync.dma_start(xpart[:, 0:512], out_b1[:])
                g = gp.tile([128, 64], F32, tag="g")
                if t == 0:
                    src = xs_own[:, 0:64]
                    nc.scalar.activation(g[:, 0:32], src[:, 0:32], AF.Sigmoid)
                    nc.scalar.activation(g[:, 48:64], src[:, 48:64], AF.Tanh)
                    nc.scalar.activation(g[:, 32:48], src[:, 32:48], AF.Sigmoid)
                else:
                    # i,f gates in their own PSUM bank so their add/sigmoid
                    # unlock as soon as the first 32 matmuls finish; the
                    # psum+xs sums land in PSUM so ACT reads the faster port
                    ps_if = gps.tile([128, 32], F32, tag="gps_if", bufs=1)
                    ps_og = gps.tile([128, 32], F32, tag="gps_og", bufs=1)
                    for m in M_IF + M_G + M_O:
                        pc, rows, wc, q, k0 = mtile_info(m)
                        dst = ps_if if pc < 32 else ps_og
                        for k in range(4):
                            nc.tensor.matmul(
                                dst[:rows, pc % 32:pc % 32 + 4],
                                whh_l[:, k * 1600 + wc: k * 1600 + wc + rows],
                                xov_[:, t - 1, k * 4:(k + 1) * 4],
                                start=(k == 0), stop=(k == 3))
                    nc.vector.tensor_add(g[:, 0:32], ps_if[:], xs_own[:, t * 64:t * 64 + 32])
                    # i,f first so c=f*c overlaps tanh(g); o off the critical path
                    nc.scalar.activation(g[:, 0:32], g[:, 0:32], AF.Sigmoid)
                    nc.vector.tensor_add(g[:, 32:64], ps_og[:], xs_own[:, t * 64 + 32:t * 64 + 64])
                    nc.scalar.activation(g[:, 48:64], g[:, 48:64], AF.Tanh)
                    nc.scalar.activation(g[:, 32:48], g[:, 32:48], AF.Sigmoid)
                nc.vector.tensor_mul(c_t[:], g[:, 16:32], c_t[:])          # f*c
                tmp = sp.tile([128, 16], F32, tag="tmp")
                nc.vector.tensor_mul(tmp[:], g[:, 0:16], g[:, 48:64])      # i*tanh(g)
                nc.vector.tensor_add(c_t[:], c_t[:], tmp[:])
                thc = sp.tile([128, 16], F32, tag="thc")
                nc.scalar.activation(thc[:], c_t[:], AF.Tanh)
                nc.vector.tensor_mul(xov_[:, t, :], g[:, 32:48], thc[:])   # h (fp16 out)

            # ---- second exchange half (t 32..63 of xout) ----
            nc.vector.tensor_scalar_mul(stg[:, 512:1024], xout[:, 512:1024],
                                        masks_sb[:, 2:3])
            nc.gpsimd.tensor_scalar_mul(stg[:, 1536:2048], xout[:, 512:1024],
                                        masks_sb[:, 3:4])
            for j in range(2):
                nc.sync.dma_start(in_b2[:][j], stg[:, j * 1024 + 512:j * 1024 + 1024])
            nc.gpsimd.collective_compute(
                "ReduceScatter", ALU.add, ins=[in_b2[:].opt()],
                outs=[out_b2[:].opt()], replica_groups=groups)
            if l < NL - 1:
                wihr_cur, wihr_next = wihr_next, (load_wihr(l + 2) if l + 2 < NL else None)
                # phase A: own-dir contribution straight from local xout
                # (wihr halves are per-core ordered [own, part]); overlaps the RS
                xs_matmul(wihr_cur,
                          [(k, xov_[:, :, k * 4:(k + 1) * 4]) for k in range(4)],
                          l + 1, True)
            else:
                # head own-side work, also overlapping the final RS
                P_own = php.tile([128, 1024], F16, tag="Pown")
                poly_eval(P_own[:], xout[:])
                Pov = P_own[:].rearrange("p (t c) -> p t c", c=16)
                psA = pps.tile([128, 256], F32, tag="pps")
                halfmat(w1_sb["ao"], Pov, psA)
                psC = pps.tile([128, 256], F32, tag="pps")
                halfmat(w1_sb["bo"], Pov, psC)
                nc.vector.tensor_scalar(A_sb[:100, :], psA[:100, :],
                                        consts_sb[:100, 5:6], None, op0=ALU.add)  # + b1
                nc.vector.tensor_copy(C_sb[:100, :], psC[:100, :])
            nc.sync.dma_start(xpart[:, 512:1024], out_b2[:])
            xpv = xpart[:].rearrange("p (t c) -> p t c", c=16)
            if l < NL - 1:
                # phase B: partner contribution, reversed in time at evac;
                # reversed t-chunks so low-t xs rows (read first) land first
                for r0 in (48, 32, 16, 0):
                    xs_matmul(wihr_cur,
                              [(4 + j, xpv[:, :, j * 4:j * 4 + 4]) for j in range(4)],
                              l + 1, False, r0, r0 + 16)
            else:
                P_part = php.tile([128, 1024], F16, tag="Ppart")
                poly_eval(P_part[:, 0:512], xpart[:, 0:512])      # half 1 lands early
                poly_eval(P_part[:, 512:1024], xpart[:, 512:1024])
                Ppv = P_part[:].rearrange("p (t c) -> p t c", c=16)
                psA2 = pps.tile([128, 256], F32, tag="pps")
                halfmat(w1_sb["ap"], Ppv, psA2)
                psC2 = pps.tile([128, 256], F32, tag="pps")
                halfmat(w1_sb["bp"], Ppv, psC2)
                for dst, ps2 in ((A_sb, psA2), (C_sb, psC2)):
                    dv = dst[:100, :].rearrange("p (t b) -> p t b", b=4)
                    pv = ps2[:100, :].rearrange("p (t b) -> p t b", b=4)
                    nc.vector.tensor_add(dv, dv, pv[:, ::-1, :])

        # ---------------- final MLP reduction ----------------
        out_sb = php.tile([128, 64], F32, tag="osb")
        Cv = C_sb[:100, :].rearrange("p (t b) -> p t b", b=4)
        ps4 = pps.tile([128, 64], F32, tag="pps")
        for a in range(32):
            hm = gp.tile([128, 256], F16, tag="hm")
            for b in range(4):
                eng = nc.vector if (a * 4 + b) % 3 else nc.gpsimd
                eng.tensor_scalar(hm[:100, b * 64:(b + 1) * 64], Cv[:, :, b],
                                  A_sb[:100, a * 4 + b:a * 4 + b + 1], 0.0,
                                  op0=ALU.add, op1=ALU.max)
            for ch in range(2):
                # out[bc, 0] = sum_k hm[k, ch*128+bc] * W2[k]
                nc.tensor.matmul(ps4[:, a * 2 + ch:a * 2 + ch + 1],
                                 hm[:100, ch * 128:(ch + 1) * 128],
                                 consts16[:100, 7:8])
        nc.vector.tensor_scalar(out_sb[:, :], ps4[:, :], consts_sb[0:128, 6:7], None,
                                op0=ALU.add)  # + b2
        nc.sync.dma_start(out_d.ap().rearrange("a (ch p) -> p a ch", p=128),
                          out_sb[:].rearrange("p (a ch) -> p a ch", ch=2))
    nc.compile()
    return nc


_CACHE = {}


def _get_program():
    if "nc" not in _CACHE:
        _CACHE["nc"] = build_program()
    return _CACHE["nc"]


def _prep_core_inputs(c, words, pos, w_emb, t_emb, Wih0, Wih_rest, Whh, bih, bhh,
                      ws, mlp_W1, mlp_b1, mlp_W2, mlp_b2):
    d, g = c // 4, c % 4
    bs = slice(4 * g, 4 * g + 4)
    # x0T: (128, 1024) f16, col = t*16 + j*4 + b  (slot-order time)
    X = np.concatenate([w_emb[words[bs]], t_emb[pos[bs]]], axis=-1)  # (4,64,400)
    if d == 1:
        X = X[:, ::-1]
    Xp = np.concatenate([X, np.zeros((4, 64, 112), X.dtype)], -1)    # pad 512
    x0T = Xp.reshape(4, 64, 4, 128).transpose(3, 1, 2, 0).reshape(128, 1024)
    whhT = np.stack([_prep_lhsT(Whh[l, d], 1) for l in range(NL)])
    wih0T = _prep_lhsT(Wih0[d], 1)

    def _ro(W):  # reorder contraction halves to [own-dir, partner-dir]
        return np.concatenate([W[:, d * 400:(d + 1) * 400],
                               W[:, (1 - d) * 400:(2 - d) * 400]], axis=1)
    wihrT = np.stack([_prep_lhsT(_ro(Wih_rest[l - 1, d]), 2) for l in range(1, NL)])
    bias = np.stack([_prep_bias(bih[l, d] + bhh[l, d]) for l in range(NL)])
    masks = np.zeros((4, 128), np.float32)
    masks[0] = float(d == 0); masks[1] = float(d == 1)
    masks[2] = float(d == 1); masks[3] = float(d == 0)
    consts = np.zeros((8, 128), np.float32)
    for i in range(5):
        consts[i] = ws[i]
    consts[5, :100] = mlp_b1
    consts[6] = mlp_b2[0]
    consts[7, :100] = mlp_W2[0]
    W1a, W1b = mlp_W1[:, :800], mlp_W1[:, 800:]
    return {
        "x0T": np.ascontiguousarray(x0T.astype(np.float16)),
        "whhT": whhT, "wih0T": wih0T, "wihrT": wihrT,
        "bias": np.ascontiguousarray(bias),
        "masks": masks, "consts": consts,
        "w1aT_own": _prep_w1h(W1a[:, d * 400:(d + 1) * 400]),
        "w1aT_part": _prep_w1h(W1a[:, (1 - d) * 400:(2 - d) * 400]),
        "w1bT_own": _prep_w1h(W1b[:, d * 400:(d + 1) * 400]),
        "w1bT_part": _prep_w1h(W1b[:, (1 - d) * 400:(2 - d) * 400]),
    }


def kernel(words_idx_tensor, pos_idx_tensor, max_length, w_emb, t_emb, Wih0, Wih_rest,
           Whh, bih, bhh, w1, w2, w3, w4, w5, mlp_W1, mlp_b1, mlp_W2, mlp_b2,
           _stats=None, _trace=False):
    words = np.asarray(words_idx_tensor)[:, :int(max_length)].astype(np.int64)
    pos = np.asarray(pos_idx_tensor)[:, :int(max_length)].astype(np.int64)
    assert words.shape == (B, L)
    args = tuple(np.asarray(x, np.float32) for x in
                 (w_emb, t_emb, Wih0, Wih_rest, Whh, bih, bhh))
    ws = [float(np.asarray(w).reshape(-1)[0]) for w in (w1, w2, w3, w4, w5)]
    mW1, mb1, mW2, mb2 = (np.asarray(mlp_W1, np.float32), np.asarray(mlp_b1, np.float32),
                          np.asarray(mlp_W2, np.float32), np.asarray(mlp_b2, np.float32))
    in_maps = [_prep_core_inputs(c, words, pos, *args, ws, mW1, mb1, mW2, mb2)
               for c in range(NCORES)]
    nc = _get_program()
    res = run_bass_kernel_spmd(nc, in_maps, list(range(NCORES)), trace=_trace)
    if _stats is not None:
        _stats["exec_time_ns"] = res.exec_time_ns
        _stats["mean_exec_time_ns"] = res.mean_exec_time_ns
        _stats["profile_json"] = res.profile_json
    out = np.zeros((L, B, L, 1), np.float32)
    ar = np.arange(32)
    ac = np.arange(L)
    for c in range(NCORES):
        d, g = c // 4, c % 4
        ch = res.results[c]["out"].reshape(32, 4, 64)
        for bl in range(4):
            if d == 0:
                out[ar[:, None], 4 * g + bl, ac[None, :], 0] = ch[:, bl, :]
            else:
                out[(63 - ar)[:, None], 4 * g + bl, (63 - ac)[None, :], 0] = ch[:, bl, :]
    return out



# revision 51
# speedup vs baseline: 1.0114x; 1.0114x over previous
"""Trainium2 Bass kernel for nn_LSTMEncoder: 5-layer bidirectional LSTM (B=16,L=64,H=400)
+ pairwise quintic-poly MLP head, algebraically collapsed.

Sharding: 8 cores = 2 directions x 4 batch-groups (B=4/core). Direction is encoded in
per-core DATA (weights/masks/index order), program is identical (SPMD).

Per-layer dir-pair exchange: masked-staging pair ReduceScatter delivering only the
PARTNER's hidden states, split in two halves — the first half fires mid-recurrence
(t 0..31 final after step 31) and hides under the remaining steps. The next layer's
input transform is split into an own-dir phase (runs during the exchange, straight
from local xout) and a partner phase (time-chunked in reverse so the rows the next
recurrence reads first are evacuated first). The MLP head computes its own-direction
poly/matmul work under the final exchange; all head matmuls run in fp16.
"""
import numpy as np
from contextlib import ExitStack

import concourse.bass as bass
import concourse.bacc as bacc
import concourse.tile as tile
from concourse import mybir
from concourse.bass_utils import run_bass_kernel_spmd

F32 = mybir.dt.float32
F16 = mybir.dt.float16
F8 = mybir.dt.float8e4
NP_F8 = mybir.dt.np(F8)
AF = mybir.ActivationFunctionType
ALU = mybir.AluOpType

H = 400
L = 64          # seq len / steps
B = 16          # total batch
BC = 4          # batch per core
NL = 5
NCORES = 8
GATE_SRC = [0, 1, 3, 2]   # q order (i,f,o,g) -> original gate block (i,f,g,o)

# ---------------- M-tile geometry ----------------
# 16 M-tiles: m<12 -> (q=m//3, k=m%3), 128 rows; m>=12 -> q=m-12, k=3, 16 rows.
def mtile_info(m):
    if m < 12:
        q, k = divmod(m, 3)
        return q * 16 + k * 4, 128, (q * 3 + k) * 128, q, k
    q = m - 12
    return q * 16 + 12, 16, 1536 + q * 16, q, 3


def _col_order():
    """order[j] = original Whh row index placed at lhsT free-col j."""
    order = []
    for q in range(4):
        for k in range(3):
            for r in range(128):
                order.append(GATE_SRC[q] * 400 + k * 128 + r)
    for q in range(4):
        for r in range(16):
            order.append(GATE_SRC[q] * 400 + 384 + r)
    return np.array(order)

COL_ORDER = _col_order()


def _prep_lhsT(W, nhalves):
    """W: (1600, D) with D = 400*nhalves. Returns (4*nhalves, 128, 1600) fp16 lhsT tiles.
    Rows (contraction) are split into nhalves halves of 400, each zero-padded to 512."""
    Wr = W[COL_ORDER, :]                       # (1600, D) reordered gate rows
    halves = []
    for s in range(nhalves):
        h = Wr[:, s * 400:(s + 1) * 400]       # (1600, 400)
        h = np.concatenate([h, np.zeros((1600, 112), h.dtype)], axis=1)  # pad to 512
        halves.append(h)
    Wp = np.concatenate(halves, axis=1)        # (1600, 512*nh)
    lhsT = Wp.T.reshape(4 * nhalves, 128, 1600)
    return np.ascontiguousarray(lhsT.astype(np.float16))


def _prep_bias(bvec):
    """(1600,) -> (16,128) f32 per-M-tile per-partition bias."""
    b = bvec[COL_ORDER]
    out = np.zeros((16, 128), np.float32)
    for m in range(16):
        pc, rows, wc, q, k = mtile_info(m)
        out[m, :rows] = b[wc:wc + rows]
    return out


def _prep_w1h(W1h):
    """W1h: (100, 400) -> (4,128,100) fp16 lhsT tiles (400 rows padded to 512)."""
    T = W1h.T  # (400, 100)
    Tp = np.concatenate([T, np.zeros((112, 100), T.dtype)], 0)
    return np.ascontiguousarray(Tp.reshape(4, 128, 100).astype(np.float16))


# ---------------- device program ----------------
def build_program():
    nc = bacc.Bacc("TRN2", target_bir_lowering=False, debug=False, num_devices=NCORES)
    dp = nc.declare_dram_parameter
    x0T_d = dp("x0T", [128, 16 * L], F16, isOutput=False)
    whh_d = dp("whhT", [NL, 4, 128, 1600], F16, isOutput=False)
    wih0_d = dp("wih0T", [4, 128, 1600], F16, isOutput=False)
    wihr_d = dp("wihrT", [NL - 1, 8, 128, 1600], F16, isOutput=False)
    bias_d = dp("bias", [NL, 16, 128], F32, isOutput=False)
    masks_d = dp("masks", [4, 128], F32, isOutput=False)
    consts_d = dp("consts", [8, 128], F32, isOutput=False)
    w1ao_d = dp("w1aT_own", [4, 128, 100], F16, isOutput=False)
    w1ap_d = dp("w1aT_part", [4, 128, 100], F16, isOutput=False)
    w1bo_d = dp("w1bT_own", [4, 128, 100], F16, isOutput=False)
    w1bp_d = dp("w1bT_part", [4, 128, 100], F16, isOutput=False)
    out_d = dp("out", [32, 256], F32, isOutput=True)

    groups = [[g, g + 4] for g in range(4)]

    with tile.TileContext(nc) as tc, ExitStack() as ctx:
        pool1 = ctx.enter_context(tc.tile_pool(name="persist", bufs=1))
        whhp = ctx.enter_context(tc.tile_pool(name="whh", bufs=2))
        wihp = ctx.enter_context(tc.tile_pool(name="wih", bufs=2))
        xsp = ctx.enter_context(tc.tile_pool(name="xs", bufs=1))
        xop = ctx.enter_context(tc.tile_pool(name="xout", bufs=2))
        xip = ctx.enter_context(tc.tile_pool(name="xin", bufs=2))
        gp = ctx.enter_context(tc.tile_pool(name="gates", bufs=3))
        sp = ctx.enter_context(tc.tile_pool(name="small", bufs=4))
        php = ctx.enter_context(tc.tile_pool(name="phase", bufs=1))
        gps = ctx.enter_context(tc.tile_pool(name="gpsum", bufs=2, space="PSUM"))
        xps = ctx.enter_context(tc.tile_pool(name="xpsum", bufs=2, space="PSUM"))
        pps = ctx.enter_context(tc.tile_pool(name="ppsum", bufs=2, space="PSUM"))
        drp = ctx.enter_context(tc.tile_pool(name="dram", bufs=2, space="DRAM"))

        # ---- persistent loads (x0T + wih0 first: they gate the first matmuls) ----
        x0T = pool1.tile([128, 16 * L], F16, tag="x0T")
        nc.sync.dma_start(x0T[:], x0T_d[:, :])
        # persistent xs buffer (memset once; garbage cols stay 0)
        xs_own = pool1.tile([128, L * 64], F32, tag="xs_own")
        nc.vector.memset(xs_own[:], 0.0)
        # zero the gate-psum slots once (garbage lanes read later must be finite 0)
        z = gps.tile([128, 32], F32, tag="gps_if", bufs=1)
        nc.vector.memset(z[:], 0.0)
        z2 = gps.tile([128, 32], F32, tag="gps_og", bufs=1)
        nc.vector.memset(z2[:], 0.0)

        def xs_matmul(lhsT_sb, pairs, lay, own, t0=0, t1=L):
            """pairs: (lhsT k-tile idx, rhs [p,t,4] view); contracts rhs[:, t0:t1].
            own=True: xs_own[t0:t1] = bias + contrib (overwrite);
            own=False: xs_own[L-t1:L-t0] += contrib reversed in t."""
            xov = xs_own[:].rearrange("p (t c) -> p t c", c=64)
            nt = t1 - t0
            for m in range(16):
                pc, rows, wc, q, k0 = mtile_info(m)
                ps = xps.tile([128, 256], F32, tag="xps")
                for i, (ki, rhs) in enumerate(pairs):
                    lw = lhsT_sb[:, ki * 1600 + wc: ki * 1600 + wc + rows]
                    nc.tensor.matmul(ps[:rows, :nt * 4], lw[:, :], rhs[:, t0:t1, :],
                                     start=(i == 0), stop=(i == len(pairs) - 1))
                pv = ps[:rows, :nt * 4].rearrange("p (t b) -> p t b", b=4)
                if own:
                    biasap = bias_sb[:rows, lay * 16 + m: lay * 16 + m + 1]
                    nc.vector.tensor_scalar(xov[:rows, t0:t1, pc:pc + 4], pv, biasap, None,
                                            op0=ALU.add)
                else:
                    nc.vector.tensor_add(xov[:rows, L - t1:L - t0, pc:pc + 4],
                                         xov[:rows, L - t1:L - t0, pc:pc + 4],
                                         pv[:, ::-1, :])

        # ---- layer 0 xs ----
        x0v = x0T[:].rearrange("p (t c) -> p t c", c=16)
        wih0_sb = wihp.tile([128, 4 * 1600], F16, tag="wih", bufs=1)
        for k in range(4):
            nc.sync.dma_start(wih0_sb[:, k * 1600:(k + 1) * 1600], wih0_d.ap()[k])
        bias_sb = pool1.tile([128, NL * 16], F32, tag="bias")
        nc.sync.dma_start(bias_sb[:], bias_d.ap().rearrange("l m p -> p (l m)"))
        masks_sb = pool1.tile([128, 4], F32, tag="masks")
        nc.sync.dma_start(masks_sb[:], masks_d.ap().rearrange("m p -> p m"))
        consts_sb = pool1.tile([128, 8], F32, tag="consts")
        nc.sync.dma_start(consts_sb[:], consts_d.ap().rearrange("m p -> p m"))
        for t0 in range(0, L, 16):  # t-chunked so the layer-0 loop starts early
            xs_matmul(wih0_sb, [(k, x0v[:, :, k * 4:(k + 1) * 4]) for k in range(4)],
                      0, True, t0, t0 + 16)

        def load_whh(l):
            t = whhp.tile([128, 4 * 1600], F16, tag="whh", name=f"whh{l}")
            nc.sync.dma_start(t[:].rearrange("p (k n) -> p k n", n=1600),
                              whh_d.ap()[l].rearrange("k p n -> p k n"))
            return t

        def load_wihr(l):
            t = wihp.tile([128, 8 * 1600], F16, tag="wihr", name=f"wihr{l}")
            nc.sync.dma_start(t[:].rearrange("p (k n) -> p k n", n=1600),
                              wihr_d.ap()[l - 1].rearrange("k p n -> p k n"))
            return t

        # m-tile emission order: i,f gate tiles first (unlocks sigmoid(i,f)
        # while g,o tiles still stream), then g, then o.
        M_IF = [0, 1, 2, 3, 4, 5, 12, 13]
        M_G = [9, 10, 11, 15]
        M_O = [6, 7, 8, 14]

        whh_l = load_whh(0)
        wihr_next = load_wihr(1)

        # head weights (per-core own/part halves, fp16); loaded during layer 0
        w1_sb = {}
        for nm, dd in (("ao", w1ao_d), ("ap", w1ap_d), ("bo", w1bo_d), ("bp", w1bp_d)):
            wt = pool1.tile([128, 400], F16, tag="w1" + nm)
            nc.sync.dma_start(wt[:].rearrange("p (k n) -> p k n", n=100),
                              dd.ap().rearrange("k p n -> p k n"))
            w1_sb[nm] = wt
        consts16 = pool1.tile([128, 8], F16, tag="c16")
        nc.vector.tensor_copy(consts16[:], consts_sb[:])

        def poly_eval(dst, x_ap):
            """dst = x*(w1 + x*(w2 + x*(w3 + x*(w4 + x*w5)))) elementwise."""
            nc.vector.tensor_scalar(dst, x_ap, consts_sb[:, 4:5], consts_sb[:, 3:4],
                                    op0=ALU.mult, op1=ALU.add)
            for wi in (2, 1, 0):
                nc.vector.tensor_mul(dst, dst, x_ap)
                nc.vector.tensor_scalar(dst, dst, consts_sb[:, wi:wi + 1], None,
                                        op0=ALU.add)
            nc.vector.tensor_mul(dst, dst, x_ap)

        def halfmat(w_sb, Pv, ps):
            for j in range(4):
                nc.tensor.matmul(ps[:100, :], w_sb[:, j * 100:(j + 1) * 100],
                                 Pv[:, :, j * 4:j * 4 + 4], start=(j == 0), stop=(j == 3))

        A_sb = php.tile([128, 256], F32, tag="A")
        C_sb = php.tile([128, 256], F16, tag="C")

        for l in range(NL):
            if l > 0:
                whh_l = load_whh(l)
            xout = xop.tile([128, 16 * L], F16, tag="xout")
            xov_ = xout[:].rearrange("p (t c) -> p t c", c=16)
            c_t = sp.tile([128, 16], F32, tag="c")
            nc.vector.memset(c_t[:], 0.0)
            # exchange split in three slices: t 0..31 fires mid-loop at step 32,
            # t 32..47 at step 48; only the small t 48..63 slice is exposed
            stg = xip.tile([128, 2048], F16, tag="stg")
            in_b1 = drp.tile([2, 128, 512], F16, tag="arin")
            out_b1 = drp.tile([128, 512], F16, tag="arout")
            in_b2 = drp.tile([2, 128, 256], F16, tag="arin")
            out_b2 = drp.tile([128, 256], F16, tag="arout")
            in_b3 = drp.tile([2, 128, 256], F16, tag="arin")
            out_b3 = drp.tile([128, 256], F16, tag="arout")
            xpart = xip.tile([128, 1024], F16, tag="xpart")
            for t in range(L):
                if t == 32:
                    nc.vector.tensor_scalar_mul(stg[:, 0:512], xout[:, 0:512],
                                                masks_sb[:, 2:3])
                    nc.gpsimd.tensor_scalar_mul(stg[:, 1024:1536], xout[:, 0:512],
                                                masks_sb[:, 3:4])
                    for j in range(2):
                        nc.sync.dma_start(in_b1[:][j], stg[:, j * 1024:j * 1024 + 512])
                    nc.gpsimd.collective_compute(
                        "ReduceScatter", ALU.add, ins=[in_b1[:].opt()],
                        outs=[out_b1[:].opt()], replica_groups=groups)
                    nc.sync.dma_start(xpart[:, 0:512], out_b1[:])
                if t == 48:
                    nc.vector.tensor_scalar_mul(stg[:, 512:768], xout[:, 512:768],
                                                masks_sb[:, 2:3])
                    nc.gpsimd.tensor_scalar_mul(stg[:, 1536:1792], xout[:, 512:768],
                                                masks_sb[:, 3:4])
                    for j in range(2):
                        nc.sync.dma_start(in_b2[:][j], stg[:, j * 1024 + 512:j * 1024 + 768])
                    nc.gpsimd.collective_compute(
                        "ReduceScatter", ALU.add, ins=[in_b2[:].opt()],
                        outs=[out_b2[:].opt()], replica_groups=groups)
                    nc.sync.dma_start(xpart[:, 512:768], out_b2[:])
                g = gp.tile([128, 64], F32, tag="g")
                if t == 0:
                    src = xs_own[:, 0:64]
                    nc.scalar.activation(g[:, 0:32], src[:, 0:32], AF.Sigmoid)
                    nc.scalar.activation(g[:, 48:64], src[:, 48:64], AF.Tanh)
                    nc.scalar.activation(g[:, 32:48], src[:, 32:48], AF.Sigmoid)
                else:
                    # i,f gates in their own PSUM bank so their add/sigmoid
                    # unlock as soon as the first 32 matmuls finish
                    ps_if = gps.tile([128, 32], F32, tag="gps_if", bufs=1)
                    ps_og = gps.tile([128, 32], F32, tag="gps_og", bufs=1)
                    for m in M_IF + M_G + M_O:
                        pc, rows, wc, q, k0 = mtile_info(m)
                        dst = ps_if if pc < 32 else ps_og
                        for k in range(4):
                            nc.tensor.matmul(
                                dst[:rows, pc % 32:pc % 32 + 4],
                                whh_l[:, k * 1600 + wc: k * 1600 + wc + rows],
                                xov_[:, t - 1, k * 4:(k + 1) * 4],
                                start=(k == 0), stop=(k == 3))
                    nc.vector.tensor_add(g[:, 0:32], ps_if[:], xs_own[:, t * 64:t * 64 + 32])
                    # i,f first so c=f*c overlaps tanh(g); o off the critical path
                    nc.scalar.activation(g[:, 0:32], g[:, 0:32], AF.Sigmoid)
                    nc.vector.tensor_add(g[:, 32:64], ps_og[:], xs_own[:, t * 64 + 32:t * 64 + 64])
                    nc.scalar.activation(g[:, 48:64], g[:, 48:64], AF.Tanh)
                    nc.scalar.activation(g[:, 32:48], g[:, 32:48], AF.Sigmoid)
                nc.vector.tensor_mul(c_t[:], g[:, 16:32], c_t[:])          # f*c
                tmp = sp.tile([128, 16], F32, tag="tmp")
                nc.vector.tensor_mul(tmp[:], g[:, 0:16], g[:, 48:64])      # i*tanh(g)
                nc.vector.tensor_add(c_t[:], c_t[:], tmp[:])
                thc = sp.tile([128, 16], F32, tag="thc")
                nc.scalar.activation(thc[:], c_t[:], AF.Tanh)
                nc.vector.tensor_mul(xov_[:, t, :], g[:, 32:48], thc[:])   # h (fp16 out)

            # ---- final exchange slice (t 48..63 of xout) ----
            nc.vector.tensor_scalar_mul(stg[:, 768:1024], xout[:, 768:1024],
                                        masks_sb[:, 2:3])
            nc.gpsimd.tensor_scalar_mul(stg[:, 1792:2048], xout[:, 768:1024],
                                        masks_sb[:, 3:4])
            for j in range(2):
                nc.sync.dma_start(in_b3[:][j], stg[:, j * 1024 + 768:j * 1024 + 1024])
            nc.gpsimd.collective_compute(
                "ReduceScatter", ALU.add, ins=[in_b3[:].opt()],
                outs=[out_b3[:].opt()], replica_groups=groups)
            if l < NL - 1:
                wihr_cur, wihr_next = wihr_next, (load_wihr(l + 2) if l + 2 < NL else None)
                # phase A: own-dir contribution straight from local xout
                # (wihr halves are per-core ordered [own, part]); overlaps the RS
                xs_matmul(wihr_cur,
                          [(k, xov_[:, :, k * 4:(k + 1) * 4]) for k in range(4)],
                          l + 1, True)
            else:
                # head own-side work, also overlapping the final RS
                P_own = php.tile([128, 1024], F16, tag="Pown")
                poly_eval(P_own[:], xout[:])
                Pov = P_own[:].rearrange("p (t c) -> p t c", c=16)
                psA = pps.tile([128, 256], F32, tag="pps")
                halfmat(w1_sb["ao"], Pov, psA)
                psC = pps.tile([128, 256], F32, tag="pps")
                halfmat(w1_sb["bo"], Pov, psC)
                nc.vector.tensor_scalar(A_sb[:100, :], psA[:100, :],
                                        consts_sb[:100, 5:6], None, op0=ALU.add)  # + b1
                nc.vector.tensor_copy(C_sb[:100, :], psC[:100, :])
            nc.sync.dma_start(xpart[:, 768:1024], out_b3[:])
            xpv = xpart[:].rearrange("p (t c) -> p t c", c=16)
            if l < NL - 1:
                # phase B: partner contribution, reversed in time at evac;
                # reversed t-chunks so low-t xs rows (read first) land first
                for r0 in (48, 32, 16, 0):
                    xs_matmul(wihr_cur,
                              [(4 + j, xpv[:, :, j * 4:j * 4 + 4]) for j in range(4)],
                              l + 1, False, r0, r0 + 16)
            else:
                P_part = php.tile([128, 1024], F16, tag="Ppart")
                poly_eval(P_part[:, 0:512], xpart[:, 0:512])      # half 1 lands early
                poly_eval(P_part[:, 512:1024], xpart[:, 512:1024])
                Ppv = P_part[:].rearrange("p (t c) -> p t c", c=16)
                psA2 = pps.tile([128, 256], F32, tag="pps")
                halfmat(w1_sb["ap"], Ppv, psA2)
                psC2 = pps.tile([128, 256], F32, tag="pps")
                halfmat(w1_sb["bp"], Ppv, psC2)
                for dst, ps2 in ((A_sb, psA2), (C_sb, psC2)):
                    dv = dst[:100, :].rearrange("p (t b) -> p t b", b=4)
                    pv = ps2[:100, :].rearrange("p (t b) -> p t b", b=4)
                    nc.vector.tensor_add(dv, dv, pv[:, ::-1, :])

        # ---------------- final MLP reduction ----------------
        out_sb = php.tile([128, 64], F32, tag="osb")
        Cv = C_sb[:100, :].rearrange("p (t b) -> p t b", b=4)
        ps4 = pps.tile([128, 64], F32, tag="pps")
        for a in range(32):
            hm = gp.tile([128, 256], F16, tag="hm")
            for b in range(4):
                eng = nc.vector if (a * 4 + b) % 3 else nc.gpsimd
                eng.tensor_scalar(hm[:100, b * 64:(b + 1) * 64], Cv[:, :, b],
                                  A_sb[:100, a * 4 + b:a * 4 + b + 1], 0.0,
                                  op0=ALU.add, op1=ALU.max)
            for ch in range(2):
                # out[bc, 0] = sum_k hm[k, ch*128+bc] * W2[k]
                nc.tensor.matmul(ps4[:, a * 2 + ch:a * 2 + ch + 1],
                                 hm[:100, ch * 128:(ch + 1) * 128],
                                 consts16[:100, 7:8])
        nc.vector.tensor_scalar(out_sb[:, :], ps4[:, :], consts_sb[0:128, 6:7], None,
                                op0=ALU.add)  # + b2
        nc.sync.dma_start(out_d.ap().rearrange("a (ch p) -> p a ch", p=128),
                          out_sb[:].rearrange("p (a ch) -> p a ch", ch=2))
    nc.compile()
    return nc


_CACHE = {}


def _get_program():
    if "nc" not in _CACHE:
        _CACHE["nc"] = build_program()
    return _CACHE["nc"]


def _prep_core_inputs(c, words, pos, w_emb, t_emb, Wih0, Wih_rest, Whh, bih, bhh,
                      ws, mlp_W1, mlp_b1, mlp_W2, mlp_b2):
    d, g = c // 4, c % 4
    bs = slice(4 * g, 4 * g + 4)
    # x0T: (128, 1024) f16, col = t*16 + j*4 + b  (slot-order time)
    X = np.concatenate([w_emb[words[bs]], t_emb[pos[bs]]], axis=-1)  # (4,64,400)
    if d == 1:
        X = X[:, ::-1]
    Xp = np.concatenate([X, np.zeros((4, 64, 112), X.dtype)], -1)    # pad 512
    x0T = Xp.reshape(4, 64, 4, 128).transpose(3, 1, 2, 0).reshape(128, 1024)
    whhT = np.stack([_prep_lhsT(Whh[l, d], 1) for l in range(NL)])
    wih0T = _prep_lhsT(Wih0[d], 1)

    def _ro(W):  # reorder contraction halves to [own-dir, partner-dir]
        return np.concatenate([W[:, d * 400:(d + 1) * 400],
                               W[:, (1 - d) * 400:(2 - d) * 400]], axis=1)
    wihrT = np.stack([_prep_lhsT(_ro(Wih_rest[l - 1, d]), 2) for l in range(1, NL)])
    bias = np.stack([_prep_bias(bih[l, d] + bhh[l, d]) for l in range(NL)])
    masks = np.zeros((4, 128), np.float32)
    masks[0] = float(d == 0); masks[1] = float(d == 1)
    masks[2] = float(d == 1); masks[3] = float(d == 0)
    consts = np.zeros((8, 128), np.float32)
    for i in range(5):
        consts[i] = ws[i]
    consts[5, :100] = mlp_b1
    consts[6] = mlp_b2[0]
    consts[7, :100] = mlp_W2[0]
    W1a, W1b = mlp_W1[:, :800], mlp_W1[:, 800:]
    return {
        "x0T": np.ascontiguousarray(x0T.astype(np.float16)),
        "whhT": whhT, "wih0T": wih0T, "wihrT": wihrT,
        "bias": np.ascontiguousarray(bias),
        "masks": masks, "consts": consts,
        "w1aT_own": _prep_w1h(W1a[:, d * 400:(d + 1) * 400]),
        "w1aT_part": _prep_w1h(W1a[:, (1 - d) * 400:(2 - d) * 400]),
        "w1bT_own": _prep_w1h(W1b[:, d * 400:(d + 1) * 400]),
        "w1bT_part": _prep_w1h(W1b[:, (1 - d) * 400:(2 - d) * 400]),
    }


def kernel(words_idx_tensor, pos_idx_tensor, max_length, w_emb, t_emb, Wih0, Wih_rest,
           Whh, bih, bhh, w1, w2, w3, w4, w5, mlp_W1, mlp_b1, mlp_W2, mlp_b2,
           _stats=None, _trace=False):
    words = np.asarray(words_idx_tensor)[:, :int(max_length)].astype(np.int64)
    pos = np.asarray(pos_idx_tensor)[:, :int(max_length)].astype(np.int64)
    assert words.shape == (B, L)
    args = tuple(np.asarray(x, np.float32) for x in
                 (w_emb, t_emb, Wih0, Wih_rest, Whh, bih, bhh))
    ws = [float(np.asarray(w).reshape(-1)[0]) for w in (w1, w2, w3, w4, w5)]
    mW1, mb1, mW2, mb2 = (np.asarray(mlp_W1, np.float32), np.asarray(mlp_b1, np.float32),
                          np.asarray(mlp_W2, np.float32), np.asarray(mlp_b2, np.float32))
    in_maps = [_prep_core_inputs(c, words, pos, *args, ws, mW1, mb1, mW2, mb2)
               for c in range(NCORES)]
    nc = _get_program()
    res = run_bass_kernel_spmd(nc, in_maps, list(range(NCORES)), trace=_trace)
    if _stats is not None:
        _stats["exec_time_ns"] = res.exec_time_ns
        _stats["mean_exec_time_ns"] = res.mean_exec_time_ns
        _stats["profile_json"] = res.profile_json
    out = np.zeros((L, B, L, 1), np.float32)
    ar = np.arange(32)
    ac = np.arange(L)
    for c in range(NCORES):
        d, g = c // 4, c % 4
        ch = res.results[c]["out"].reshape(32, 4, 64)
        for bl in range(4):
            if d == 0:
                out[ar[:, None], 4 * g + bl, ac[None, :], 0] = ch[:, bl, :]
            else:
                out[(63 - ar)[:, None], 4 * g + bl, (63 - ac)[None, :], 0] = ch[:, bl, :]
    return out



# revision 53
# speedup vs baseline: 1.0155x; 1.0041x over previous
"""Trainium2 Bass kernel for nn_LSTMEncoder: 5-layer bidirectional LSTM (B=16,L=64,H=400)
+ pairwise quintic-poly MLP head, algebraically collapsed.

Sharding: 8 cores = 2 directions x 4 batch-groups (B=4/core). Direction is encoded in
per-core DATA (weights/masks/index order), program is identical (SPMD).

Per-layer dir-pair exchange: masked-staging pair ReduceScatter delivering only the
PARTNER's hidden states, split in three slices — t 0..31 fires at step 32 and
t 32..47 at step 48, both hiding under the remaining recurrence; only the small
final slice (t 48..63) is exposed at the layer boundary. The next layer's
input transform is split into an own-dir phase (runs during the exchange, straight
from local xout) and a partner phase (time-chunked in reverse so the rows the next
recurrence reads first are evacuated first). The MLP head computes its own-direction
poly/matmul work under the final exchange; all head matmuls run in fp16.
"""
import numpy as np
from contextlib import ExitStack

import concourse.bass as bass
import concourse.bacc as bacc
import concourse.tile as tile
from concourse import mybir
from concourse.bass_utils import run_bass_kernel_spmd

F32 = mybir.dt.float32
F16 = mybir.dt.float16
F8 = mybir.dt.float8e4
NP_F8 = mybir.dt.np(F8)
AF = mybir.ActivationFunctionType
ALU = mybir.AluOpType

H = 400
L = 64          # seq len / steps
B = 16          # total batch
BC = 4          # batch per core
NL = 5
NCORES = 8
GATE_SRC = [0, 1, 3, 2]   # q order (i,f,o,g) -> original gate block (i,f,g,o)

# ---------------- M-tile geometry ----------------
# 16 M-tiles: m<12 -> (q=m//3, k=m%3), 128 rows; m>=12 -> q=m-12, k=3, 16 rows.
def mtile_info(m):
    if m < 12:
        q, k = divmod(m, 3)
        return q * 16 + k * 4, 128, (q * 3 + k) * 128, q, k
    q = m - 12
    return q * 16 + 12, 16, 1536 + q * 16, q, 3


def _col_order():
    """order[j] = original Whh row index placed at lhsT free-col j."""
    order = []
    for q in range(4):
        for k in range(3):
            for r in range(128):
                order.append(GATE_SRC[q] * 400 + k * 128 + r)
    for q in range(4):
        for r in range(16):
            order.append(GATE_SRC[q] * 400 + 384 + r)
    return np.array(order)

COL_ORDER = _col_order()


def _prep_lhsT(W, nhalves):
    """W: (1600, D) with D = 400*nhalves. Returns (4*nhalves, 128, 1600) fp16 lhsT tiles.
    Rows (contraction) are split into nhalves halves of 400, each zero-padded to 512."""
    Wr = W[COL_ORDER, :]                       # (1600, D) reordered gate rows
    halves = []
    for s in range(nhalves):
        h = Wr[:, s * 400:(s + 1) * 400]       # (1600, 400)
        h = np.concatenate([h, np.zeros((1600, 112), h.dtype)], axis=1)  # pad to 512
        halves.append(h)
    Wp = np.concatenate(halves, axis=1)        # (1600, 512*nh)
    lhsT = Wp.T.reshape(4 * nhalves, 128, 1600)
    return np.ascontiguousarray(lhsT.astype(np.float16))


def _prep_bias(bvec):
    """(1600,) -> (16,128) f32 per-M-tile per-partition bias."""
    b = bvec[COL_ORDER]
    out = np.zeros((16, 128), np.float32)
    for m in range(16):
        pc, rows, wc, q, k = mtile_info(m)
        out[m, :rows] = b[wc:wc + rows]
    return out


def _prep_w1h(W1h):
    """W1h: (100, 400) -> (4,128,100) fp16 lhsT tiles (400 rows padded to 512)."""
    T = W1h.T  # (400, 100)
    Tp = np.concatenate([T, np.zeros((112, 100), T.dtype)], 0)
    return np.ascontiguousarray(Tp.reshape(4, 128, 100).astype(np.float16))


# ---------------- device program ----------------
def build_program():
    nc = bacc.Bacc("TRN2", target_bir_lowering=False, debug=False, num_devices=NCORES)
    dp = nc.declare_dram_parameter
    x0T_d = dp("x0T", [128, 16 * L], F16, isOutput=False)
    whh_d = dp("whhT", [NL, 4, 128, 1600], F16, isOutput=False)
    wih0_d = dp("wih0T", [4, 128, 1600], F16, isOutput=False)
    wihr_d = dp("wihrT", [NL - 1, 8, 128, 1600], F16, isOutput=False)
    bias_d = dp("bias", [NL, 16, 128], F32, isOutput=False)
    masks_d = dp("masks", [4, 128], F32, isOutput=False)
    consts_d = dp("consts", [8, 128], F32, isOutput=False)
    w1ao_d = dp("w1aT_own", [4, 128, 100], F16, isOutput=False)
    w1ap_d = dp("w1aT_part", [4, 128, 100], F16, isOutput=False)
    w1bo_d = dp("w1bT_own", [4, 128, 100], F16, isOutput=False)
    w1bp_d = dp("w1bT_part", [4, 128, 100], F16, isOutput=False)
    out_d = dp("out", [32, 256], F32, isOutput=True)

    groups = [[g, g + 4] for g in range(4)]

    with tile.TileContext(nc) as tc, ExitStack() as ctx:
        pool1 = ctx.enter_context(tc.tile_pool(name="persist", bufs=1))
        whhp = ctx.enter_context(tc.tile_pool(name="whh", bufs=2))
        wihp = ctx.enter_context(tc.tile_pool(name="wih", bufs=2))
        xsp = ctx.enter_context(tc.tile_pool(name="xs", bufs=1))
        xop = ctx.enter_context(tc.tile_pool(name="xout", bufs=2))
        xip = ctx.enter_context(tc.tile_pool(name="xin", bufs=2))
        gp = ctx.enter_context(tc.tile_pool(name="gates", bufs=3))
        sp = ctx.enter_context(tc.tile_pool(name="small", bufs=4))
        php = ctx.enter_context(tc.tile_pool(name="phase", bufs=1))
        gps = ctx.enter_context(tc.tile_pool(name="gpsum", bufs=2, space="PSUM"))
        xps = ctx.enter_context(tc.tile_pool(name="xpsum", bufs=2, space="PSUM"))
        pps = ctx.enter_context(tc.tile_pool(name="ppsum", bufs=2, space="PSUM"))
        drp = ctx.enter_context(tc.tile_pool(name="dram", bufs=2, space="DRAM"))

        # ---- persistent loads (x0T + wih0 first: they gate the first matmuls) ----
        x0T = pool1.tile([128, 16 * L], F16, tag="x0T")
        nc.sync.dma_start(x0T[:], x0T_d[:, :])
        # persistent xs buffer (memset once; garbage cols stay 0)
        xs_own = pool1.tile([128, L * 64], F32, tag="xs_own")
        nc.vector.memset(xs_own[:], 0.0)
        # zero the gate-psum slots once (garbage lanes read later must be finite 0)
        z = gps.tile([128, 32], F32, tag="gps_if", bufs=1)
        nc.vector.memset(z[:], 0.0)
        z2 = gps.tile([128, 32], F32, tag="gps_og", bufs=1)
        nc.vector.memset(z2[:], 0.0)

        def xs_matmul(lhsT_sb, pairs, lay, own, t0=0, t1=L):
            """pairs: (lhsT k-tile idx, rhs [p,t,4] view); contracts rhs[:, t0:t1].
            own=True: xs_own[t0:t1] = bias + contrib (overwrite);
            own=False: xs_own[L-t1:L-t0] += contrib reversed in t."""
            xov = xs_own[:].rearrange("p (t c) -> p t c", c=64)
            nt = t1 - t0
            for m in range(16):
                pc, rows, wc, q, k0 = mtile_info(m)
                ps = xps.tile([128, 256], F32, tag="xps")
                for i, (ki, rhs) in enumerate(pairs):
                    lw = lhsT_sb[:, ki * 1600 + wc: ki * 1600 + wc + rows]
                    nc.tensor.matmul(ps[:rows, :nt * 4], lw[:, :], rhs[:, t0:t1, :],
                                     start=(i == 0), stop=(i == len(pairs) - 1))
                pv = ps[:rows, :nt * 4].rearrange("p (t b) -> p t b", b=4)
                if own:
                    biasap = bias_sb[:rows, lay * 16 + m: lay * 16 + m + 1]
                    nc.vector.tensor_scalar(xov[:rows, t0:t1, pc:pc + 4], pv, biasap, None,
                                            op0=ALU.add)
                else:
                    nc.vector.tensor_add(xov[:rows, L - t1:L - t0, pc:pc + 4],
                                         xov[:rows, L - t1:L - t0, pc:pc + 4],
                                         pv[:, ::-1, :])

        # ---- layer 0 xs ----
        x0v = x0T[:].rearrange("p (t c) -> p t c", c=16)
        wih0_sb = wihp.tile([128, 4 * 1600], F16, tag="wih", bufs=1)
        for k in range(4):
            nc.sync.dma_start(wih0_sb[:, k * 1600:(k + 1) * 1600], wih0_d.ap()[k])
        bias_sb = pool1.tile([128, NL * 16], F32, tag="bias")
        nc.sync.dma_start(bias_sb[:], bias_d.ap().rearrange("l m p -> p (l m)"))
        masks_sb = pool1.tile([128, 4], F32, tag="masks")
        nc.sync.dma_start(masks_sb[:], masks_d.ap().rearrange("m p -> p m"))
        consts_sb = pool1.tile([128, 8], F32, tag="consts")
        nc.sync.dma_start(consts_sb[:], consts_d.ap().rearrange("m p -> p m"))
        for t0 in range(0, L, 16):  # t-chunked so the layer-0 loop starts early
            xs_matmul(wih0_sb, [(k, x0v[:, :, k * 4:(k + 1) * 4]) for k in range(4)],
                      0, True, t0, t0 + 16)

        def load_whh(l):
            t = whhp.tile([128, 4 * 1600], F16, tag="whh", name=f"whh{l}")
            nc.sync.dma_start(t[:].rearrange("p (k n) -> p k n", n=1600),
                              whh_d.ap()[l].rearrange("k p n -> p k n"))
            return t

        def load_wihr(l):
            t = wihp.tile([128, 8 * 1600], F16, tag="wihr", name=f"wihr{l}")
            nc.sync.dma_start(t[:].rearrange("p (k n) -> p k n", n=1600),
                              wihr_d.ap()[l - 1].rearrange("k p n -> p k n"))
            return t

        # m-tile emission order: i,f gate tiles first (unlocks sigmoid(i,f)
        # while g,o tiles still stream), then g, then o.
        M_IF = [0, 1, 2, 3, 4, 5, 12, 13]
        M_G = [9, 10, 11, 15]
        M_O = [6, 7, 8, 14]

        whh_l = load_whh(0)
        wihr_next = load_wihr(1)

        # head weights (per-core own/part halves, fp16); loaded during layer 0
        w1_sb = {}
        for nm, dd in (("ao", w1ao_d), ("ap", w1ap_d), ("bo", w1bo_d), ("bp", w1bp_d)):
            wt = pool1.tile([128, 400], F16, tag="w1" + nm)
            nc.sync.dma_start(wt[:].rearrange("p (k n) -> p k n", n=100),
                              dd.ap().rearrange("k p n -> p k n"))
            w1_sb[nm] = wt
        consts16 = pool1.tile([128, 8], F16, tag="c16")
        nc.vector.tensor_copy(consts16[:], consts_sb[:])

        def poly_eval(dst, x_ap):
            """dst = x*(w1 + x*(w2 + x*(w3 + x*(w4 + x*w5)))) elementwise."""
            nc.vector.tensor_scalar(dst, x_ap, consts_sb[:, 4:5], consts_sb[:, 3:4],
                                    op0=ALU.mult, op1=ALU.add)
            for wi in (2, 1, 0):
                nc.vector.tensor_mul(dst, dst, x_ap)
                nc.vector.tensor_scalar(dst, dst, consts_sb[:, wi:wi + 1], None,
                                        op0=ALU.add)
            nc.vector.tensor_mul(dst, dst, x_ap)

        def halfmat(w_sb, Pv, ps, t0=0, t1=L):
            for j in range(4):
                nc.tensor.matmul(ps[:100, t0 * 4:t1 * 4], w_sb[:, j * 100:(j + 1) * 100],
                                 Pv[:, t0:t1, j * 4:j * 4 + 4], start=(j == 0), stop=(j == 3))

        A_sb = php.tile([128, 256], F32, tag="A")
        C_sb = php.tile([128, 256], F16, tag="C")

        for l in range(NL):
            if l > 0:
                whh_l = load_whh(l)
            xout = xop.tile([128, 16 * L], F16, tag="xout")
            xov_ = xout[:].rearrange("p (t c) -> p t c", c=16)
            c_t = sp.tile([128, 16], F32, tag="c")
            nc.vector.memset(c_t[:], 0.0)
            # exchange split in three slices: t 0..31 fires mid-loop at step 32,
            # t 32..47 at step 48; only the small t 48..63 slice is exposed
            stg = xip.tile([128, 2048], F16, tag="stg")
            in_b1 = drp.tile([2, 128, 512], F16, tag="arin")
            out_b1 = drp.tile([128, 512], F16, tag="arout")
            in_b2 = drp.tile([2, 128, 256], F16, tag="arin")
            out_b2 = drp.tile([128, 256], F16, tag="arout")
            in_b3 = drp.tile([2, 128, 256], F16, tag="arin")
            out_b3 = drp.tile([128, 256], F16, tag="arout")
            xpart = xip.tile([128, 1024], F16, tag="xpart")
            for t in range(L):
                if t == 32:
                    nc.vector.tensor_scalar_mul(stg[:, 0:512], xout[:, 0:512],
                                                masks_sb[:, 2:3])
                    nc.gpsimd.tensor_scalar_mul(stg[:, 1024:1536], xout[:, 0:512],
                                                masks_sb[:, 3:4])
                    nc.sync.dma_start(in_b1[:].rearrange("s p c -> p s c"),
                                      stg[:].rearrange("p (s c) -> p s c", c=1024)[:, :, 0:512])
                    nc.gpsimd.collective_compute(
                        "ReduceScatter", ALU.add, ins=[in_b1[:].opt()],
                        outs=[out_b1[:].opt()], replica_groups=groups)
                    nc.sync.dma_start(xpart[:, 0:512], out_b1[:])
                if t == 48:
                    nc.vector.tensor_scalar_mul(stg[:, 512:768], xout[:, 512:768],
                                                masks_sb[:, 2:3])
                    nc.gpsimd.tensor_scalar_mul(stg[:, 1536:1792], xout[:, 512:768],
                                                masks_sb[:, 3:4])
                    nc.sync.dma_start(in_b2[:].rearrange("s p c -> p s c"),
                                      stg[:].rearrange("p (s c) -> p s c", c=1024)[:, :, 512:768])
                    nc.gpsimd.collective_compute(
                        "ReduceScatter", ALU.add, ins=[in_b2[:].opt()],
                        outs=[out_b2[:].opt()], replica_groups=groups)
                    nc.sync.dma_start(xpart[:, 512:768], out_b2[:])
                g = gp.tile([128, 64], F32, tag="g")
                if t == 0:
                    src = xs_own[:, 0:64]
                    nc.scalar.activation(g[:, 0:32], src[:, 0:32], AF.Sigmoid)
                    nc.scalar.activation(g[:, 48:64], src[:, 48:64], AF.Tanh)
                    nc.scalar.activation(g[:, 32:48], src[:, 32:48], AF.Sigmoid)
                else:
                    # i,f gates in their own PSUM bank so their add/sigmoid
                    # unlock as soon as the first 32 matmuls finish
                    ps_if = gps.tile([128, 32], F32, tag="gps_if", bufs=1)
                    ps_og = gps.tile([128, 32], F32, tag="gps_og", bufs=1)
                    for m in M_IF + M_G + M_O:
                        pc, rows, wc, q, k0 = mtile_info(m)
                        dst = ps_if if pc < 32 else ps_og
                        for k in range(4):
                            nc.tensor.matmul(
                                dst[:rows, pc % 32:pc % 32 + 4],
                                whh_l[:, k * 1600 + wc: k * 1600 + wc + rows],
                                xov_[:, t - 1, k * 4:(k + 1) * 4],
                                start=(k == 0), stop=(k == 3))
                    nc.vector.tensor_add(g[:, 0:32], ps_if[:], xs_own[:, t * 64:t * 64 + 32])
                    # i,f first so c=f*c overlaps tanh(g); o off the critical path
                    nc.scalar.activation(g[:, 0:32], g[:, 0:32], AF.Sigmoid)
                    nc.vector.tensor_add(g[:, 32:64], ps_og[:], xs_own[:, t * 64 + 32:t * 64 + 64])
                    nc.scalar.activation(g[:, 48:64], g[:, 48:64], AF.Tanh)
                    nc.scalar.activation(g[:, 32:48], g[:, 32:48], AF.Sigmoid)
                nc.vector.tensor_mul(c_t[:], g[:, 16:32], c_t[:])          # f*c
                tmp = sp.tile([128, 16], F32, tag="tmp")
                nc.vector.tensor_mul(tmp[:], g[:, 0:16], g[:, 48:64])      # i*tanh(g)
                nc.vector.tensor_add(c_t[:], c_t[:], tmp[:])
                thc = sp.tile([128, 16], F32, tag="thc")
                nc.scalar.activation(thc[:], c_t[:], AF.Tanh)
                nc.vector.tensor_mul(xov_[:, t, :], g[:, 32:48], thc[:])   # h (fp16 out)

            # ---- final exchange slice (t 48..63 of xout) ----
            nc.vector.tensor_scalar_mul(stg[:, 768:1024], xout[:, 768:1024],
                                        masks_sb[:, 2:3])
            nc.gpsimd.tensor_scalar_mul(stg[:, 1792:2048], xout[:, 768:1024],
                                        masks_sb[:, 3:4])
            nc.sync.dma_start(in_b3[:].rearrange("s p c -> p s c"),
                              stg[:].rearrange("p (s c) -> p s c", c=1024)[:, :, 768:1024])
            nc.gpsimd.collective_compute(
                "ReduceScatter", ALU.add, ins=[in_b3[:].opt()],
                outs=[out_b3[:].opt()], replica_groups=groups)
            if l < NL - 1:
                wihr_cur, wihr_next = wihr_next, (load_wihr(l + 2) if l + 2 < NL else None)
                # phase A: own-dir contribution straight from local xout
                # (wihr halves are per-core ordered [own, part]); overlaps the RS
                xs_matmul(wihr_cur,
                          [(k, xov_[:, :, k * 4:(k + 1) * 4]) for k in range(4)],
                          l + 1, True)
            else:
                # head own-side work, also overlapping the final RS
                P_own = php.tile([128, 1024], F16, tag="Pown")
                poly_eval(P_own[:], xout[:])
                Pov = P_own[:].rearrange("p (t c) -> p t c", c=16)
                psA = pps.tile([128, 256], F32, tag="pps")
                halfmat(w1_sb["ao"], Pov, psA)
                psC = pps.tile([128, 256], F32, tag="pps")
                halfmat(w1_sb["bo"], Pov, psC)
                nc.vector.tensor_scalar(A_sb[:100, :], psA[:100, :],
                                        consts_sb[:100, 5:6], None, op0=ALU.add)  # + b1
                nc.vector.tensor_copy(C_sb[:100, :], psC[:100, :])
            nc.sync.dma_start(xpart[:, 768:1024], out_b3[:])
            xpv = xpart[:].rearrange("p (t c) -> p t c", c=16)
            if l < NL - 1:
                # phase B: partner contribution, reversed in time at evac;
                # reversed t-chunks so low-t xs rows (read first) land first
                for r0 in (48, 32, 16, 0):
                    xs_matmul(wihr_cur,
                              [(4 + j, xpv[:, :, j * 4:j * 4 + 4]) for j in range(4)],
                              l + 1, False, r0, r0 + 16)
            else:
                P_part = php.tile([128, 1024], F16, tag="Ppart")
                poly_eval(P_part[:, 0:512], xpart[:, 0:512])      # slice 1: mid-loop
                poly_eval(P_part[:, 512:768], xpart[:, 512:768])  # slice 2: step 48
                poly_eval(P_part[:, 768:1024], xpart[:, 768:1024])
                Ppv = P_part[:].rearrange("p (t c) -> p t c", c=16)
                psA2 = pps.tile([128, 256], F32, tag="pps")
                psC2 = pps.tile([128, 256], F32, tag="pps")
                for t0, t1 in ((0, 48), (48, 64)):  # only t>=48 waits the last RS
                    halfmat(w1_sb["ap"], Ppv, psA2, t0, t1)
                    halfmat(w1_sb["bp"], Ppv, psC2, t0, t1)
                for dst, ps2 in ((A_sb, psA2), (C_sb, psC2)):
                    dv = dst[:100, :].rearrange("p (t b) -> p t b", b=4)
                    pv = ps2[:100, :].rearrange("p (t b) -> p t b", b=4)
                    nc.vector.tensor_add(dv, dv, pv[:, ::-1, :])

        # ---------------- final MLP reduction ----------------
        out_sb = php.tile([128, 64], F32, tag="osb")
        Cv = C_sb[:100, :].rearrange("p (t b) -> p t b", b=4)
        ps4 = pps.tile([128, 64], F32, tag="pps")
        for a in range(32):
            hm = gp.tile([128, 256], F16, tag="hm")
            for b in range(4):
                eng = nc.vector if (a * 4 + b) % 3 else nc.gpsimd
                eng.tensor_scalar(hm[:100, b * 64:(b + 1) * 64], Cv[:, :, b],
                                  A_sb[:100, a * 4 + b:a * 4 + b + 1], 0.0,
                                  op0=ALU.add, op1=ALU.max)
            for ch in range(2):
                # out[bc, 0] = sum_k hm[k, ch*128+bc] * W2[k]
                nc.tensor.matmul(ps4[:, a * 2 + ch:a * 2 + ch + 1],
                                 hm[:100, ch * 128:(ch + 1) * 128],
                                 consts16[:100, 7:8])
        nc.vector.tensor_scalar(out_sb[:, :], ps4[:, :], consts_sb[0:128, 6:7], None,
                                op0=ALU.add)  # + b2
        nc.sync.dma_start(out_d.ap().rearrange("a (ch p) -> p a ch", p=128),
                          out_sb[:].rearrange("p (a ch) -> p a ch", ch=2))
    nc.compile()
    return nc


_CACHE = {}


def _get_program():
    if "nc" not in _CACHE:
        _CACHE["nc"] = build_program()
    return _CACHE["nc"]


def _prep_core_inputs(c, words, pos, w_emb, t_emb, Wih0, Wih_rest, Whh, bih, bhh,
                      ws, mlp_W1, mlp_b1, mlp_W2, mlp_b2):
    d, g = c // 4, c % 4
    bs = slice(4 * g, 4 * g + 4)
    # x0T: (128, 1024) f16, col = t*16 + j*4 + b  (slot-order time)
    X = np.concatenate([w_emb[words[bs]], t_emb[pos[bs]]], axis=-1)  # (4,64,400)
    if d == 1:
        X = X[:, ::-1]
    Xp = np.concatenate([X, np.zeros((4, 64, 112), X.dtype)], -1)    # pad 512
    x0T = Xp.reshape(4, 64, 4, 128).transpose(3, 1, 2, 0).reshape(128, 1024)
    whhT = np.stack([_prep_lhsT(Whh[l, d], 1) for l in range(NL)])
    wih0T = _prep_lhsT(Wih0[d], 1)

    def _ro(W):  # reorder contraction halves to [own-dir, partner-dir]
        return np.concatenate([W[:, d * 400:(d + 1) * 400],
                               W[:, (1 - d) * 400:(2 - d) * 400]], axis=1)
    wihrT = np.stack([_prep_lhsT(_ro(Wih_rest[l - 1, d]), 2) for l in range(1, NL)])
    bias = np.stack([_prep_bias(bih[l, d] + bhh[l, d]) for l in range(NL)])
    masks = np.zeros((4, 128), np.float32)
    masks[0] = float(d == 0); masks[1] = float(d == 1)
    masks[2] = float(d == 1); masks[3] = float(d == 0)
    consts = np.zeros((8, 128), np.float32)
    for i in range(5):
        consts[i] = ws[i]
    consts[5, :100] = mlp_b1
    consts[6] = mlp_b2[0]
    consts[7, :100] = mlp_W2[0]
    W1a, W1b = mlp_W1[:, :800], mlp_W1[:, 800:]
    return {
        "x0T": np.ascontiguousarray(x0T.astype(np.float16)),
        "whhT": whhT, "wih0T": wih0T, "wihrT": wihrT,
        "bias": np.ascontiguousarray(bias),
        "masks": masks, "consts": consts,
        "w1aT_own": _prep_w1h(W1a[:, d * 400:(d + 1) * 400]),
        "w1aT_part": _prep_w1h(W1a[:, (1 - d) * 400:(2 - d) * 400]),
        "w1bT_own": _prep_w1h(W1b[:, d * 400:(d + 1) * 400]),
        "w1bT_part": _prep_w1h(W1b[:, (1 - d) * 400:(2 - d) * 400]),
    }


def kernel(words_idx_tensor, pos_idx_tensor, max_length, w_emb, t_emb, Wih0, Wih_rest,
           Whh, bih, bhh, w1, w2, w3, w4, w5, mlp_W1, mlp_b1, mlp_W2, mlp_b2,
           _stats=None, _trace=False):
    words = np.asarray(words_idx_tensor)[:, :int(max_length)].astype(np.int64)
    pos = np.asarray(pos_idx_tensor)[:, :int(max_length)].astype(np.int64)
    assert words.shape == (B, L)
    args = tuple(np.asarray(x, np.float32) for x in
                 (w_emb, t_emb, Wih0, Wih_rest, Whh, bih, bhh))
    ws = [float(np.asarray(w).reshape(-1)[0]) for w in (w1, w2, w3, w4, w5)]
    mW1, mb1, mW2, mb2 = (np.asarray(mlp_W1, np.float32), np.asarray(mlp_b1, np.float32),
                          np.asarray(mlp_W2, np.float32), np.asarray(mlp_b2, np.float32))
    in_maps = [_prep_core_inputs(c, words, pos, *args, ws, mW1, mb1, mW2, mb2)
               for c in range(NCORES)]
    nc = _get_program()
    res = run_bass_kernel_spmd(nc, in_maps, list(range(NCORES)), trace=_trace)
    if _stats is not None:
        _stats["exec_time_ns"] = res.exec_time_ns
        _stats["mean_exec_time_ns"] = res.mean_exec_time_ns
        _stats["profile_json"] = res.profile_json
    out = np.zeros((L, B, L, 1), np.float32)
    ar = np.arange(32)
    ac = np.arange(L)
    for c in range(NCORES):
        d, g = c // 4, c % 4
        ch = res.results[c]["out"].reshape(32, 4, 64)
        for bl in range(4):
            if d == 0:
                out[ar[:, None], 4 * g + bl, ac[None, :], 0] = ch[:, bl, :]
            else:
                out[(63 - ar)[:, None], 4 * g + bl, (63 - ac)[None, :], 0] = ch[:, bl, :]
    return out



# revision 57
# speedup vs baseline: 1.1822x; 1.1641x over previous
"""Trainium2 Bass kernel for nn_LSTMEncoder: 5-layer bidirectional LSTM (B=16,L=64,H=400)
+ pairwise quintic-poly MLP head, algebraically collapsed.

Sharding: 8 cores = 2 directions x 4 batch-groups (B=4/core). Direction is encoded in
per-core DATA (weights/masks/index order), program is identical (SPMD).

Per-layer dir-pair exchange: masked-staging pair ReduceScatter delivering only the
PARTNER's hidden states, split in three slices — t 0..31 fires at step 32 and
t 32..47 at step 48, both hiding under the remaining recurrence; only the small
final slice (t 48..63) is exposed at the layer boundary. The next layer's
input transform is split into an own-dir phase (runs during the exchange, straight
from local xout) and a partner phase (time-chunked in reverse so the rows the next
recurrence reads first are evacuated first). The MLP head computes its own-direction
poly/matmul work under the final exchange; all head matmuls run in fp16.
"""
import numpy as np
from contextlib import ExitStack

import concourse.bass as bass
import concourse.bacc as bacc
import concourse.tile as tile
from concourse import mybir
from concourse.bass_utils import run_bass_kernel_spmd

F32 = mybir.dt.float32
F16 = mybir.dt.float16
F8 = mybir.dt.float8e4
NP_F8 = mybir.dt.np(F8)
AF = mybir.ActivationFunctionType
ALU = mybir.AluOpType

H = 400
L = 64          # seq len / steps
B = 16          # total batch
BC = 4          # batch per core
NL = 5
NCORES = 8
GATE_SRC = [0, 1, 3, 2]   # q order (i,f,o,g) -> original gate block (i,f,g,o)

# ---------------- M-tile geometry ----------------
# 16 M-tiles: m<12 -> (q=m//3, k=m%3), 128 rows; m>=12 -> q=m-12, k=3, 16 rows.
def mtile_info(m):
    if m < 12:
        q, k = divmod(m, 3)
        return q * 16 + k * 4, 128, (q * 3 + k) * 128, q, k
    q = m - 12
    return q * 16 + 12, 16, 1536 + q * 16, q, 3


def _col_order():
    """order[j] = original Whh row index placed at lhsT free-col j."""
    order = []
    for q in range(4):
        for k in range(3):
            for r in range(128):
                order.append(GATE_SRC[q] * 400 + k * 128 + r)
    for q in range(4):
        for r in range(16):
            order.append(GATE_SRC[q] * 400 + 384 + r)
    return np.array(order)

COL_ORDER = _col_order()


def _prep_lhsT(W, nhalves):
    """W: (1600, D) with D = 400*nhalves. Returns (4*nhalves, 128, 1600) fp16 lhsT tiles.
    Rows (contraction) are split into nhalves halves of 400, each zero-padded to 512."""
    Wr = W[COL_ORDER, :]                       # (1600, D) reordered gate rows
    halves = []
    for s in range(nhalves):
        h = Wr[:, s * 400:(s + 1) * 400]       # (1600, 400)
        h = np.concatenate([h, np.zeros((1600, 112), h.dtype)], axis=1)  # pad to 512
        halves.append(h)
    Wp = np.concatenate(halves, axis=1)        # (1600, 512*nh)
    lhsT = Wp.T.reshape(4 * nhalves, 128, 1600)
    return np.ascontiguousarray(lhsT.astype(np.float16))


def _prep_bias(bvec):
    """(1600,) -> (16,128) f32 per-M-tile per-partition bias."""
    b = bvec[COL_ORDER]
    out = np.zeros((16, 128), np.float32)
    for m in range(16):
        pc, rows, wc, q, k = mtile_info(m)
        out[m, :rows] = b[wc:wc + rows]
    return out


def _prep_w1h(W1h):
    """W1h: (100, 400) -> (4,128,100) fp16 lhsT tiles (400 rows padded to 512)."""
    T = W1h.T  # (400, 100)
    Tp = np.concatenate([T, np.zeros((112, 100), T.dtype)], 0)
    return np.ascontiguousarray(Tp.reshape(4, 128, 100).astype(np.float16))


# ---------------- device program ----------------
def build_program():
    nc = bacc.Bacc("TRN2", target_bir_lowering=False, debug=False, num_devices=NCORES)
    dp = nc.declare_dram_parameter
    x0T_d = dp("x0T", [128, 16 * L], F16, isOutput=False)
    whh_d = dp("whhT", [NL, 4, 128, 1600], F16, isOutput=False)
    wih0_d = dp("wih0T", [4, 128, 1600], F16, isOutput=False)
    wihr_d = dp("wihrT", [NL - 1, 8, 128, 1600], F16, isOutput=False)
    bias_d = dp("bias", [NL, 16, 128], F32, isOutput=False)
    masks_d = dp("masks", [4, 128], F32, isOutput=False)
    consts_d = dp("consts", [8, 128], F32, isOutput=False)
    ident_d = dp("ident", [128, 128], F32, isOutput=False)
    w1ao_d = dp("w1aT_own", [4, 128, 100], F16, isOutput=False)
    w1ap_d = dp("w1aT_part", [4, 128, 100], F16, isOutput=False)
    w1bo_d = dp("w1bT_own", [4, 128, 100], F16, isOutput=False)
    w1bp_d = dp("w1bT_part", [4, 128, 100], F16, isOutput=False)
    out_d = dp("out", [32, 256], F32, isOutput=True)

    groups = [[g, g + 4] for g in range(4)]

    with tile.TileContext(nc) as tc, ExitStack() as ctx:
        pool1 = ctx.enter_context(tc.tile_pool(name="persist", bufs=1))
        whhp = ctx.enter_context(tc.tile_pool(name="whh", bufs=2))
        wihp = ctx.enter_context(tc.tile_pool(name="wih", bufs=2))
        xsp = ctx.enter_context(tc.tile_pool(name="xs", bufs=1))
        xop = ctx.enter_context(tc.tile_pool(name="xout", bufs=2))
        xip = ctx.enter_context(tc.tile_pool(name="xin", bufs=2))
        gp = ctx.enter_context(tc.tile_pool(name="gates", bufs=3))
        sp = ctx.enter_context(tc.tile_pool(name="small", bufs=4))
        php = ctx.enter_context(tc.tile_pool(name="phase", bufs=1))
        gps = ctx.enter_context(tc.tile_pool(name="gpsum", bufs=2, space="PSUM"))
        xps = ctx.enter_context(tc.tile_pool(name="xpsum", bufs=2, space="PSUM"))
        pps = ctx.enter_context(tc.tile_pool(name="ppsum", bufs=2, space="PSUM"))
        drp = ctx.enter_context(tc.tile_pool(name="dram", bufs=2, space="DRAM"))

        # ---- persistent loads (x0T + wih0 first: they gate the first matmuls) ----
        x0T = pool1.tile([128, 16 * L], F16, tag="x0T")
        nc.sync.dma_start(x0T[:], x0T_d[:, :])
        # persistent xs buffer (memset once; garbage cols stay 0)
        xs_own = pool1.tile([128, L * 64], F32, tag="xs_own")
        nc.vector.memset(xs_own[:], 0.0)
        # zero the gate-psum slots once (garbage lanes read later must be finite 0)
        z = gps.tile([128, 32], F32, tag="gps_if", bufs=1)
        nc.vector.memset(z[:], 0.0)
        z2 = gps.tile([128, 32], F32, tag="gps_og", bufs=1)
        nc.vector.memset(z2[:], 0.0)

        def xs_matmul(lhsT_sb, pairs, lay, own, t0=0, t1=L):
            """pairs: (lhsT k-tile idx, rhs [p,t,4] view); contracts rhs[:, t0:t1].
            own=True: xs_own[t0:t1] = bias + contrib (overwrite);
            own=False: xs_own[L-t1:L-t0] += contrib reversed in t."""
            xov = xs_own[:].rearrange("p (t c) -> p t c", c=64)
            nt = t1 - t0
            for m in range(16):
                pc, rows, wc, q, k0 = mtile_info(m)
                ps = xps.tile([128, 256], F32, tag="xps")
                for i, (ki, rhs) in enumerate(pairs):
                    lw = lhsT_sb[:, ki * 1600 + wc: ki * 1600 + wc + rows]
                    nc.tensor.matmul(ps[:rows, :nt * 4], lw[:, :], rhs[:, t0:t1, :],
                                     start=(i == 0), stop=(i == len(pairs) - 1))
                pv = ps[:rows, :nt * 4].rearrange("p (t b) -> p t b", b=4)
                if own:
                    biasap = bias_sb[:rows, lay * 16 + m: lay * 16 + m + 1]
                    nc.vector.tensor_scalar(xov[:rows, t0:t1, pc:pc + 4], pv, biasap, None,
                                            op0=ALU.add)
                else:
                    nc.vector.tensor_add(xov[:rows, L - t1:L - t0, pc:pc + 4],
                                         xov[:rows, L - t1:L - t0, pc:pc + 4],
                                         pv[:, ::-1, :])

        # ---- layer 0 xs ----
        x0v = x0T[:].rearrange("p (t c) -> p t c", c=16)
        wih0_sb = wihp.tile([128, 4 * 1600], F16, tag="wih", bufs=1)
        for k in range(4):
            nc.sync.dma_start(wih0_sb[:, k * 1600:(k + 1) * 1600], wih0_d.ap()[k])
        bias_sb = pool1.tile([128, NL * 16], F32, tag="bias")
        nc.sync.dma_start(bias_sb[:], bias_d.ap().rearrange("l m p -> p (l m)"))
        masks_sb = pool1.tile([128, 4], F32, tag="masks")
        nc.sync.dma_start(masks_sb[:], masks_d.ap().rearrange("m p -> p m"))
        consts_sb = pool1.tile([128, 8], F32, tag="consts")
        nc.sync.dma_start(consts_sb[:], consts_d.ap().rearrange("m p -> p m"))
        ident_sb = pool1.tile([128, 128], F32, tag="ident")
        nc.sync.dma_start(ident_sb[:], ident_d.ap())
        for t0 in range(0, L, 16):  # t-chunked so the layer-0 loop starts early
            xs_matmul(wih0_sb, [(k, x0v[:, :, k * 4:(k + 1) * 4]) for k in range(4)],
                      0, True, t0, t0 + 16)

        def load_whh(l):
            t = whhp.tile([128, 4 * 1600], F16, tag="whh", name=f"whh{l}")
            nc.sync.dma_start(t[:].rearrange("p (k n) -> p k n", n=1600),
                              whh_d.ap()[l].rearrange("k p n -> p k n"))
            return t

        def load_wihr(l):
            t = wihp.tile([128, 8 * 1600], F16, tag="wihr", name=f"wihr{l}")
            nc.sync.dma_start(t[:].rearrange("p (k n) -> p k n", n=1600),
                              wihr_d.ap()[l - 1].rearrange("k p n -> p k n"))
            return t

        # m-tile emission order: i,f gate tiles first (unlocks sigmoid(i,f)
        # while g,o tiles still stream), then g, then o.
        M_IF = [0, 1, 2, 3, 4, 5, 12, 13]
        M_G = [9, 10, 11, 15]
        M_O = [6, 7, 8, 14]

        whh_l = load_whh(0)
        wihr_next = load_wihr(1)

        # head weights (per-core own/part halves, fp16); loaded during layer 0
        w1_sb = {}
        for nm, dd in (("ao", w1ao_d), ("ap", w1ap_d), ("bo", w1bo_d), ("bp", w1bp_d)):
            wt = pool1.tile([128, 400], F16, tag="w1" + nm)
            nc.sync.dma_start(wt[:].rearrange("p (k n) -> p k n", n=100),
                              dd.ap().rearrange("k p n -> p k n"))
            w1_sb[nm] = wt
        consts16 = pool1.tile([128, 8], F16, tag="c16")
        nc.vector.tensor_copy(consts16[:], consts_sb[:])

        def poly_eval(dst, x_ap):
            """dst = x*(w1 + x*(w2 + x*(w3 + x*(w4 + x*w5)))) elementwise."""
            nc.vector.tensor_scalar(dst, x_ap, consts_sb[:, 4:5], consts_sb[:, 3:4],
                                    op0=ALU.mult, op1=ALU.add)
            for wi in (2, 1, 0):
                nc.vector.tensor_mul(dst, dst, x_ap)
                nc.vector.tensor_scalar(dst, dst, consts_sb[:, wi:wi + 1], None,
                                        op0=ALU.add)
            nc.vector.tensor_mul(dst, dst, x_ap)

        def halfmat(w_sb, Pv, ps, t0=0, t1=L):
            for j in range(4):
                nc.tensor.matmul(ps[:100, t0 * 4:t1 * 4], w_sb[:, j * 100:(j + 1) * 100],
                                 Pv[:, t0:t1, j * 4:j * 4 + 4], start=(j == 0), stop=(j == 3))

        A_sb = php.tile([128, 256], F32, tag="A")
        C_sb = php.tile([128, 256], F16, tag="C")

        for l in range(NL):
            if l > 0:
                whh_l = load_whh(l)
            xout = xop.tile([128, 16 * L], F16, tag="xout")
            xov_ = xout[:].rearrange("p (t c) -> p t c", c=16)
            c_t = sp.tile([128, 16], F32, tag="c")
            nc.vector.memset(c_t[:], 0.0)
            # exchange split in three slices: t 0..31 fires mid-loop at step 32,
            # t 32..47 at step 48; only the small t 48..63 slice is exposed
            stg = xip.tile([128, 2048], F16, tag="stg")
            in_b1 = drp.tile([2, 128, 512], F16, tag="arin")
            out_b1 = drp.tile([128, 512], F16, tag="arout")
            in_b2 = drp.tile([2, 128, 256], F16, tag="arin")
            out_b2 = drp.tile([128, 256], F16, tag="arout")
            in_b3 = drp.tile([2, 128, 256], F16, tag="arin")
            out_b3 = drp.tile([128, 256], F16, tag="arout")
            xpart = xip.tile([128, 1024], F16, tag="xpart")
            for t in range(L):
                if t == 32:
                    nc.vector.tensor_scalar_mul(stg[:, 0:512], xout[:, 0:512],
                                                masks_sb[:, 2:3])
                    nc.gpsimd.tensor_scalar_mul(stg[:, 1024:1536], xout[:, 0:512],
                                                masks_sb[:, 3:4])
                    nc.sync.dma_start(in_b1[:].rearrange("s p c -> p s c"),
                                      stg[:].rearrange("p (s c) -> p s c", c=1024)[:, :, 0:512])
                    nc.gpsimd.collective_compute(
                        "ReduceScatter", ALU.add, ins=[in_b1[:].opt()],
                        outs=[out_b1[:].opt()], replica_groups=groups)
                    nc.sync.dma_start(xpart[:, 0:512], out_b1[:])
                if t == 48:
                    nc.vector.tensor_scalar_mul(stg[:, 512:768], xout[:, 512:768],
                                                masks_sb[:, 2:3])
                    nc.gpsimd.tensor_scalar_mul(stg[:, 1536:1792], xout[:, 512:768],
                                                masks_sb[:, 3:4])
                    nc.sync.dma_start(in_b2[:].rearrange("s p c -> p s c"),
                                      stg[:].rearrange("p (s c) -> p s c", c=1024)[:, :, 512:768])
                    nc.gpsimd.collective_compute(
                        "ReduceScatter", ALU.add, ins=[in_b2[:].opt()],
                        outs=[out_b2[:].opt()], replica_groups=groups)
                    nc.sync.dma_start(xpart[:, 512:768], out_b2[:])
                g = gp.tile([128, 64], F32, tag="g")
                if t == 0:
                    src = xs_own[:, 0:64]
                    nc.scalar.activation(g[:, 0:32], src[:, 0:32], AF.Sigmoid)
                    nc.scalar.activation(g[:, 48:64], src[:, 48:64], AF.Tanh)
                    nc.scalar.activation(g[:, 32:48], src[:, 32:48], AF.Sigmoid)
                else:
                    # xs is injected into each gate psum bank by an identity
                    # matmul as the group's start=True member (a real PE write,
                    # so has_written is set legitimately); the weight matmuls
                    # accumulate onto it and the activations read PSUM directly
                    # -- no psum+xs add in the serial chain.
                    ps_if = gps.tile([128, 32], F32, tag="gps_if", bufs=1)
                    ps_og = gps.tile([128, 32], F32, tag="gps_og", bufs=1)
                    nc.tensor.matmul(ps_if[:], ident_sb[:],
                                     xs_own[:, t * 64:t * 64 + 32],
                                     start=True, stop=False, skip_group_check=True)
                    nc.tensor.matmul(ps_og[:], ident_sb[:],
                                     xs_own[:, t * 64 + 32:t * 64 + 64],
                                     start=True, stop=False, skip_group_check=True)
                    for m in M_IF + M_G + M_O:
                        pc, rows, wc, q, k0 = mtile_info(m)
                        dst = ps_if if pc < 32 else ps_og
                        for k in range(4):
                            nc.tensor.matmul(
                                dst[:rows, pc % 32:pc % 32 + 4],
                                whh_l[:, k * 1600 + wc: k * 1600 + wc + rows],
                                xov_[:, t - 1, k * 4:(k + 1) * 4],
                                start=False, stop=(k == 3 and m in (13, 14)),
                                skip_group_check=True)
                    # i,f first so c=f*c overlaps tanh(g); o off the critical path
                    nc.scalar.activation(g[:, 0:32], ps_if[:], AF.Sigmoid)
                    nc.scalar.activation(g[:, 48:64], ps_og[:, 16:32], AF.Tanh)
                    nc.scalar.activation(g[:, 32:48], ps_og[:, 0:16], AF.Sigmoid)
                nc.vector.tensor_mul(c_t[:], g[:, 16:32], c_t[:])          # f*c
                tmp = sp.tile([128, 16], F32, tag="tmp")
                nc.vector.tensor_mul(tmp[:], g[:, 0:16], g[:, 48:64])      # i*tanh(g)
                nc.vector.tensor_add(c_t[:], c_t[:], tmp[:])
                thc = sp.tile([128, 16], F32, tag="thc")
                nc.scalar.activation(thc[:], c_t[:], AF.Tanh)
                nc.vector.tensor_mul(xov_[:, t, :], g[:, 32:48], thc[:])   # h (fp16 out)

            # ---- final exchange slice (t 48..63 of xout) ----
            nc.vector.tensor_scalar_mul(stg[:, 768:1024], xout[:, 768:1024],
                                        masks_sb[:, 2:3])
            nc.gpsimd.tensor_scalar_mul(stg[:, 1792:2048], xout[:, 768:1024],
                                        masks_sb[:, 3:4])
            nc.sync.dma_start(in_b3[:].rearrange("s p c -> p s c"),
                              stg[:].rearrange("p (s c) -> p s c", c=1024)[:, :, 768:1024])
            nc.gpsimd.collective_compute(
                "ReduceScatter", ALU.add, ins=[in_b3[:].opt()],
                outs=[out_b3[:].opt()], replica_groups=groups)
            if l < NL - 1:
                wihr_cur, wihr_next = wihr_next, (load_wihr(l + 2) if l + 2 < NL else None)
                # phase A: own-dir contribution straight from local xout
                # (wihr halves are per-core ordered [own, part]); overlaps the RS
                xs_matmul(wihr_cur,
                          [(k, xov_[:, :, k * 4:(k + 1) * 4]) for k in range(4)],
                          l + 1, True)
            else:
                # head own-side work, also overlapping the final RS
                P_own = php.tile([128, 1024], F16, tag="Pown")
                poly_eval(P_own[:], xout[:])
                Pov = P_own[:].rearrange("p (t c) -> p t c", c=16)
                psA = pps.tile([128, 256], F32, tag="pps")
                halfmat(w1_sb["ao"], Pov, psA)
                psC = pps.tile([128, 256], F32, tag="pps")
                halfmat(w1_sb["bo"], Pov, psC)
                nc.vector.tensor_scalar(A_sb[:100, :], psA[:100, :],
                                        consts_sb[:100, 5:6], None, op0=ALU.add)  # + b1
                nc.vector.tensor_copy(C_sb[:100, :], psC[:100, :])
            nc.sync.dma_start(xpart[:, 768:1024], out_b3[:])
            xpv = xpart[:].rearrange("p (t c) -> p t c", c=16)
            if l < NL - 1:
                # phase B: partner contribution, reversed in time at evac;
                # reversed t-chunks so low-t xs rows (read first) land first
                for r0 in (48, 32, 16, 0):
                    xs_matmul(wihr_cur,
                              [(4 + j, xpv[:, :, j * 4:j * 4 + 4]) for j in range(4)],
                              l + 1, False, r0, r0 + 16)
            else:
                P_part = php.tile([128, 1024], F16, tag="Ppart")
                poly_eval(P_part[:, 0:512], xpart[:, 0:512])      # slice 1: mid-loop
                poly_eval(P_part[:, 512:768], xpart[:, 512:768])  # slice 2: step 48
                poly_eval(P_part[:, 768:1024], xpart[:, 768:1024])
                Ppv = P_part[:].rearrange("p (t c) -> p t c", c=16)
                psA2 = pps.tile([128, 256], F32, tag="pps")
                psC2 = pps.tile([128, 256], F32, tag="pps")
                for t0, t1 in ((0, 48), (48, 64)):  # only t>=48 waits the last RS
                    halfmat(w1_sb["ap"], Ppv, psA2, t0, t1)
                    halfmat(w1_sb["bp"], Ppv, psC2, t0, t1)
                for dst, ps2 in ((A_sb, psA2), (C_sb, psC2)):
                    dv = dst[:100, :].rearrange("p (t b) -> p t b", b=4)
                    pv = ps2[:100, :].rearrange("p (t b) -> p t b", b=4)
                    nc.vector.tensor_add(dv, dv, pv[:, ::-1, :])

        # ---------------- final MLP reduction ----------------
        out_sb = php.tile([128, 64], F32, tag="osb")
        Cv = C_sb[:100, :].rearrange("p (t b) -> p t b", b=4)
        ps4 = pps.tile([128, 64], F32, tag="pps")
        for a in range(32):
            hm = gp.tile([128, 256], F16, tag="hm")
            for b in range(4):
                eng = nc.vector if (a * 4 + b) % 3 else nc.gpsimd
                eng.tensor_scalar(hm[:100, b * 64:(b + 1) * 64], Cv[:, :, b],
                                  A_sb[:100, a * 4 + b:a * 4 + b + 1], 0.0,
                                  op0=ALU.add, op1=ALU.max)
            for ch in range(2):
                # out[bc, 0] = sum_k hm[k, ch*128+bc] * W2[k]
                nc.tensor.matmul(ps4[:, a * 2 + ch:a * 2 + ch + 1],
                                 hm[:100, ch * 128:(ch + 1) * 128],
                                 consts16[:100, 7:8])
        nc.vector.tensor_scalar(out_sb[:, :], ps4[:, :], consts_sb[0:128, 6:7], None,
                                op0=ALU.add)  # + b2
        nc.sync.dma_start(out_d.ap().rearrange("a (ch p) -> p a ch", p=128),
                          out_sb[:].rearrange("p (a ch) -> p a ch", ch=2))
    nc.compile()
    return nc


_CACHE = {}


def _get_program():
    if "nc" not in _CACHE:
        _CACHE["nc"] = build_program()
    return _CACHE["nc"]


def _prep_core_inputs(c, words, pos, w_emb, t_emb, Wih0, Wih_rest, Whh, bih, bhh,
                      ws, mlp_W1, mlp_b1, mlp_W2, mlp_b2):
    d, g = c // 4, c % 4
    bs = slice(4 * g, 4 * g + 4)
    # x0T: (128, 1024) f16, col = t*16 + j*4 + b  (slot-order time)
    X = np.concatenate([w_emb[words[bs]], t_emb[pos[bs]]], axis=-1)  # (4,64,400)
    if d == 1:
        X = X[:, ::-1]
    Xp = np.concatenate([X, np.zeros((4, 64, 112), X.dtype)], -1)    # pad 512
    x0T = Xp.reshape(4, 64, 4, 128).transpose(3, 1, 2, 0).reshape(128, 1024)
    whhT = np.stack([_prep_lhsT(Whh[l, d], 1) for l in range(NL)])
    wih0T = _prep_lhsT(Wih0[d], 1)

    def _ro(W):  # reorder contraction halves to [own-dir, partner-dir]
        return np.concatenate([W[:, d * 400:(d + 1) * 400],
                               W[:, (1 - d) * 400:(2 - d) * 400]], axis=1)
    wihrT = np.stack([_prep_lhsT(_ro(Wih_rest[l - 1, d]), 2) for l in range(1, NL)])
    bias = np.stack([_prep_bias(bih[l, d] + bhh[l, d]) for l in range(NL)])
    masks = np.zeros((4, 128), np.float32)
    masks[0] = float(d == 0); masks[1] = float(d == 1)
    masks[2] = float(d == 1); masks[3] = float(d == 0)
    consts = np.zeros((8, 128), np.float32)
    for i in range(5):
        consts[i] = ws[i]
    consts[5, :100] = mlp_b1
    consts[6] = mlp_b2[0]
    consts[7, :100] = mlp_W2[0]
    W1a, W1b = mlp_W1[:, :800], mlp_W1[:, 800:]
    return {
        "x0T": np.ascontiguousarray(x0T.astype(np.float16)),
        "whhT": whhT, "wih0T": wih0T, "wihrT": wihrT,
        "bias": np.ascontiguousarray(bias),
        "masks": masks, "consts": consts,
        "ident": np.eye(128, dtype=np.float32),
        "w1aT_own": _prep_w1h(W1a[:, d * 400:(d + 1) * 400]),
        "w1aT_part": _prep_w1h(W1a[:, (1 - d) * 400:(2 - d) * 400]),
        "w1bT_own": _prep_w1h(W1b[:, d * 400:(d + 1) * 400]),
        "w1bT_part": _prep_w1h(W1b[:, (1 - d) * 400:(2 - d) * 400]),
    }


def kernel(words_idx_tensor, pos_idx_tensor, max_length, w_emb, t_emb, Wih0, Wih_rest,
           Whh, bih, bhh, w1, w2, w3, w4, w5, mlp_W1, mlp_b1, mlp_W2, mlp_b2,
           _stats=None, _trace=False):
    words = np.asarray(words_idx_tensor)[:, :int(max_length)].astype(np.int64)
    pos = np.asarray(pos_idx_tensor)[:, :int(max_length)].astype(np.int64)
    assert words.shape == (B, L)
    args = tuple(np.asarray(x, np.float32) for x in
                 (w_emb, t_emb, Wih0, Wih_rest, Whh, bih, bhh))
    ws = [float(np.asarray(w).reshape(-1)[0]) for w in (w1, w2, w3, w4, w5)]
    mW1, mb1, mW2, mb2 = (np.asarray(mlp_W1, np.float32), np.asarray(mlp_b1, np.float32),
                          np.asarray(mlp_W2, np.float32), np.asarray(mlp_b2, np.float32))
    in_maps = [_prep_core_inputs(c, words, pos, *args, ws, mW1, mb1, mW2, mb2)
               for c in range(NCORES)]
    nc = _get_program()
    res = run_bass_kernel_spmd(nc, in_maps, list(range(NCORES)), trace=_trace)
    if _stats is not None:
        _stats["exec_time_ns"] = res.exec_time_ns
        _stats["mean_exec_time_ns"] = res.mean_exec_time_ns
        _stats["profile_json"] = res.profile_json
    out = np.zeros((L, B, L, 1), np.float32)
    ar = np.arange(32)
    ac = np.arange(L)
    for c in range(NCORES):
        d, g = c // 4, c % 4
        ch = res.results[c]["out"].reshape(32, 4, 64)
        for bl in range(4):
            if d == 0:
                out[ar[:, None], 4 * g + bl, ac[None, :], 0] = ch[:, bl, :]
            else:
                out[(63 - ar)[:, None], 4 * g + bl, (63 - ac)[None, :], 0] = ch[:, bl, :]
    return out

